# revision 1
# baseline (speedup 1.0000x reference)
"""MoE transformer layer on 8 Trainium2 NeuronCores.

Strategy:
  Launch 1 (attention block): shard by (batch, seq-half) -> 8 cores.
    Each core holds all 1024 tokens of its batch (for K/V) with its own
    512 query tokens ordered first, computes LN1 -> MHA -> residual ->
    LN2 entirely in a transposed [E, token] layout (E on partitions, so
    every bias/LN-gain is a per-partition scalar and no transposes are
    needed anywhere). Outputs x2T and h2T per core.
  Host: top-2 gating (softmax over 8 logits, renormalized), builds the
    per-expert token batches (all-to-all dispatch done on host).
  Launch 2 (expert FFN): expert-parallel, core e owns expert e.
    toksT [E, C] -> gelu(w1.T @ toks + b1) -> w2.T @ h + b2 -> outT.
  Host: scatter-add combine with gate weights + residual.
"""

import numpy as np

import concourse.bass as bass
import concourse.tile as tile
from concourse import bacc, mybir
from concourse.bass_utils import run_bass_kernel_spmd

S, B, E = 1024, 4, 1024
H, DH = 16, 64
F, NE = 4096, 8
N = S * B
NCORES = 8
Q = 512          # query tokens per core
KV = 1024        # key/value tokens per core (full batch-b sequence)
C = 1280         # expert capacity (max expert load for seed-0 inputs is 1076)
CT = [(0, 512), (512, 512), (1024, 256)]  # (offset, width) token tiles in launch 2
ET = E // 128    # 8
FT = F // 128    # 32

f32 = mybir.dt.float32
f32r = mybir.dt.float32r
bf16 = mybir.dt.bfloat16
AF = mybir.ActivationFunctionType
ALU = mybir.AluOpType

_GELU = AF.Gelu  # patchable for CoreSim (which lacks Gelu)

_programs = {}


def _bcast_dram(ap2d, nparts):
    """Partition-broadcast DMA source: read a [D,1] dram slice into [nparts, D]."""
    return bass.AP(tensor=ap2d.tensor, offset=ap2d.offset, ap=[[0, nparts]] + ap2d.ap)


def _build_launch1():
    nc = bacc.Bacc("TRN2", target_bir_lowering=False, debug=False, num_devices=NCORES)

    xT_d = nc.dram_tensor("xT", [E, KV], f32, kind="ExternalInput").ap()
    wqkvT_d = nc.dram_tensor("wqkvT", [E, 3 * E], f32, kind="ExternalInput").ap()
    bqkv_d = nc.dram_tensor("bqkv", [3 * E, 1], f32, kind="ExternalInput").ap()
    woT_d = nc.dram_tensor("woT", [E, E], f32, kind="ExternalInput").ap()
    bo_d = nc.dram_tensor("bo", [E, 1], f32, kind="ExternalInput").ap()
    g1_d = nc.dram_tensor("g1", [E, 1], f32, kind="ExternalInput").ap()
    b1_d = nc.dram_tensor("b1", [E, 1], f32, kind="ExternalInput").ap()
    g2_d = nc.dram_tensor("g2", [E, 1], f32, kind="ExternalInput").ap()
    b2_d = nc.dram_tensor("b2", [E, 1], f32, kind="ExternalInput").ap()
    sel_d = nc.dram_tensor("sel", [4 * 8, 128], f32, kind="ExternalInput").ap()
    ident_d = nc.dram_tensor("ident", [128, 128], f32, kind="ExternalInput").ap()
    x2T_d = nc.dram_tensor("x2T", [E, Q], f32, kind="ExternalOutput").ap()
    h2T_d = nc.dram_tensor("h2T", [E, Q], f32, kind="ExternalOutput").ap()

    tc_ctx = tile.TileContext(nc)
    with tc_ctx as tc:
        consts = tc.alloc_tile_pool(name="consts", bufs=1)
        statp = tc.alloc_tile_pool(name="stat", bufs=1)
        bcp = tc.alloc_tile_pool(name="bc", bufs=1)
        sqp = tc.alloc_tile_pool(name="sqp", bufs=2)
        wsp = tc.alloc_tile_pool(name="wstream", bufs=4)
        otp = tc.alloc_tile_pool(name="otp", bufs=1)
        outp = tc.alloc_tile_pool(name="outp", bufs=1)
        pmm = tc.alloc_tile_pool(name="pmm", bufs=2, space="PSUM")
        pmm2 = tc.alloc_tile_pool(name="pmm2", bufs=2, space="PSUM")
        pav = tc.alloc_tile_pool(name="pav", bufs=2, space="PSUM")

        ones128 = consts.tile([128, 1], f32r, tag="ones128")
        nc.vector.memset(ones128[:].bitcast(f32), 1.0)
        ones1 = consts.tile([1, 128], f32r, tag="ones1")
        nc.vector.memset(ones1[:].bitcast(f32), 1.0)
        eps = consts.tile([1, 1], f32, tag="eps")
        nc.vector.memset(eps[:], 1e-5)

        ident = consts.tile([128, 128], f32r, tag="ident")
        nc.sync.dma_start(out=ident[:], in_=ident_d.bitcast(f32r))

        # head-pair selector matrices (host-supplied): sel[d4].T @ recipA
        # broadcasts head-row 2*d4 to partitions 0..63, 2*d4+1 to 64..127
        sel_tiles = []
        for d4 in range(4):
            st = consts.tile([8, 128], f32r, tag=f"sel{d4}", name=f"sel{d4}")
            nc.sync.dma_start(out=st[:],
                              in_=sel_d[d4 * 8:(d4 + 1) * 8, :].bitcast(f32r))
            sel_tiles.append(st)

        def ppar(dram, k, tag):
            t = consts.tile([128, k], f32, tag=tag, name=tag)
            nc.sync.dma_start(out=t[:], in_=dram.rearrange("(a p) o -> p (a o)", p=128))
            return t

        g1_sb = ppar(g1_d, ET, "g1c")
        b1_sb = ppar(b1_d, ET, "b1c")
        g2_sb = ppar(g2_d, ET, "g2c")
        b2_sb = ppar(b2_d, ET, "b2c")
        bo_sb = ppar(bo_d, ET, "boc")
        bqkv_sb = ppar(bqkv_d, 24, "bqkvc")

        # ---------- LN helper: stats along partitions via ones-matmul ----------
        def ln_stats(src_tiles, ncols, tagpfx):
            s1 = statp.tile([1, KV], f32r, tag="s1row", name=f"{tagpfx}_s1")
            s2 = statp.tile([1, KV], f32r, tag="s2row", name=f"{tagpfx}_s2")
            tmp = statp.tile([1, KV], f32r, tag="tmprow", name=f"{tagpfx}_tmp")
            for h in range(ncols // 512):
                cs = slice(h * 512, (h + 1) * 512)
                p1 = pmm.tile([1, 512], f32, tag="mm", name=f"{tagpfx}_p1_{h}")
                for i in range(ET):
                    nc.tensor.matmul(p1[:], ones128[:],
                                     src_tiles[i][:, cs],
                                     start=(i == 0), stop=(i == ET - 1))
                nc.vector.tensor_copy(out=s1[:, cs], in_=p1[:])
                p2 = pmm.tile([1, 512], f32, tag="mm", name=f"{tagpfx}_p2_{h}")
                for i in range(ET):
                    sq = sqp.tile([128, 512], f32r, tag="sq", name=f"{tagpfx}_sq_{h}_{i}")
                    nc.vector.tensor_mul(sq[:], src_tiles[i][:, cs], src_tiles[i][:, cs])
                    nc.tensor.matmul(p2[:], ones128[:], sq[:],
                                     start=(i == 0), stop=(i == ET - 1))
                nc.vector.tensor_copy(out=s2[:, cs], in_=p2[:])
            cs = slice(0, ncols)
            # s1 <- mean ; s2 <- E[x^2] ; tmp <- mean^2 ; s2 <- var
            nc.vector.tensor_scalar(out=s1[:, cs], in0=s1[:, cs], scalar1=1.0 / E,
                                    scalar2=None, op0=ALU.mult)
            nc.vector.tensor_scalar(out=s2[:, cs], in0=s2[:, cs], scalar1=1.0 / E,
                                    scalar2=None, op0=ALU.mult)
            nc.vector.tensor_mul(tmp[:, cs], s1[:, cs], s1[:, cs])
            nc.vector.tensor_sub(s2[:, cs], s2[:, cs], tmp[:, cs])
            # s2 <- rstd = exp(-0.5*ln(var+eps))
            nc.scalar.activation(out=tmp[:, cs], in_=s2[:, cs], func=AF.Ln,
                                 bias=eps[:], scale=1.0)
            nc.scalar.activation(out=s2[:, cs], in_=tmp[:, cs], func=AF.Exp, scale=-0.5)
            # tmp <- -mean  (apply order: y = (x - mean) * rstd, then gain/bias on ACT)
            nc.vector.tensor_scalar(out=tmp[:, cs], in0=s1[:, cs], scalar1=-1.0,
                                    scalar2=None, op0=ALU.mult)
            return s2, tmp

        def bcast_rows(rowap, ncols, tagname):
            dst = bcp.tile([128, ncols], f32, tag=tagname, name=f"bc_{tagname}")
            for h in range(ncols // 512):
                cs = slice(h * 512, (h + 1) * 512)
                pb = pmm.tile([128, 512], f32, tag="mm", name=f"bc_{tagname}_{h}")
                nc.tensor.matmul(pb[:], ones1[:], rowap[:, cs],
                                 start=True, stop=True)
                nc.vector.tensor_copy(out=dst[:, cs], in_=pb[:])
            return dst

        # ---------- phase 1: load x, LN1 ----------
        xqp = tc.alloc_tile_pool(name="xqp", bufs=1)
        lxp = tc.alloc_tile_pool(name="lxp", bufs=1)
        xp = tc.alloc_tile_pool(name="xp", bufs=1)

        x_sb = []
        for i in range(ET):
            t = xp.tile([128, KV], f32r, tag=f"x{i}", name=f"x_sb{i}")
            nc.sync.dma_start(out=t[:], in_=xT_d[i * 128:(i + 1) * 128, :].bitcast(f32r))
            x_sb.append(t)

        rstd1, beta1 = ln_stats(x_sb, KV, "ln1")
        aB1 = bcast_rows(rstd1, KV, "aB1")

        lx = []
        xq = []
        for i in range(ET):
            t = lxp.tile([128, KV], f32r, tag=f"lx{i}", name=f"lx{i}")
            for hf in range(2):
                cs = slice(hf * 512, (hf + 1) * 512)
                pl = pmm.tile([128, 512], f32, tag="mm", name=f"pl_{i}_{hf}")
                nc.tensor.matmul(pl[:], ident[:], x_sb[i][:, cs],
                                 start=True, stop=False)
                nc.tensor.matmul(pl[:], ones1[:], beta1[:, cs],
                                 start=False, stop=True)
                nc.vector.tensor_mul(t[:, cs], pl[:], aB1[:, cs])
            nc.scalar.activation(out=t[:], in_=t[:], func=AF.Identity,
                                 scale=g1_sb[:, i:i + 1], bias=b1_sb[:, i:i + 1])
            lx.append(t)
            tq = xqp.tile([128, Q], f32, tag=f"xq{i}", name=f"xq{i}")
            nc.gpsimd.tensor_copy(out=tq[:], in_=x_sb[i][:, 0:Q])
            xq.append(tq)
        xp.release()

        # ---------- phase 2: attention ----------
        wvp = tc.alloc_tile_pool(name="wvp", bufs=1)
        vp = tc.alloc_tile_pool(name="vp", bufs=1)
        qkp = tc.alloc_tile_pool(name="qkp", bufs=2)
        attnp = tc.alloc_tile_pool(name="attnp", bufs=3)

        oT = []
        for i in range(ET):
            oT.append(otp.tile([128, Q], f32r, tag=f"oT{i}", name=f"oT{i}"))

        for half in range(2):
            # V projection for this half (8 heads), token-major with ones column
            wv = []
            for kt in range(ET):
                wt = wvp.tile([128, 512], f32r, tag=f"wv{kt}", name=f"wv_{half}_{kt}", bufs=2)
                nc.sync.dma_start(
                    out=wt[:],
                    in_=wqkvT_d[kt * 128:(kt + 1) * 128,
                                2 * E + half * 512: 2 * E + (half + 1) * 512].bitcast(f32r))
                wv.append(wt)
            bvB = bcp.tile([128, 512], f32, tag="bvB", name=f"bvB_{half}", bufs=2)
            nc.sync.dma_start(
                out=bvB[:],
                in_=_bcast_dram(bqkv_d[2 * E + half * 512: 2 * E + (half + 1) * 512, :], 128))
            v_sb = []
            for tt in range(ET):
                pv = pmm.tile([128, 512], f32, tag="mm", name=f"pv_{half}_{tt}")
                for kt in range(ET):
                    nc.tensor.matmul(pv[:],
                                     lx[kt][:, tt * 128:(tt + 1) * 128],
                                     wv[kt][:],
                                     start=(kt == 0), stop=(kt == ET - 1))
                vt = vp.tile([128, 8 * 65], f32r, tag=f"v{tt}", name=f"v_{half}_{tt}")
                nc.vector.tensor_add(
                    vt[:].rearrange("p (h d) -> p h d", h=8)[:, :, 0:64],
                    pv[:].rearrange("p (h d) -> p h d", h=8),
                    bvB[:].rearrange("p (h d) -> p h d", h=8))
                nc.vector.memset(vt[:].rearrange("p (h d) -> p h d", h=8)[:, :, 64:65].bitcast(f32), 1.0)
                v_sb.append(vt)

            denAll = statp.tile([8, Q], f32, tag="den", name=f"den_{half}", bufs=2)

            for dt in range(half * 4, half * 4 + 4):
                # qT [128, Q] for d-rows dt*128..
                pq = pmm.tile([128, Q], f32, tag="mm", name=f"pq_{dt}")
                for kt in range(ET):
                    wt = wsp.tile([128, 128], f32r, tag="wq", name=f"wq_{dt}_{kt}")
                    nc.sync.dma_start(
                        out=wt[:],
                        in_=wqkvT_d[kt * 128:(kt + 1) * 128, dt * 128:(dt + 1) * 128].bitcast(f32r))
                    nc.tensor.matmul(pq[:], wt[:],
                                     lx[kt][:, 0:Q],
                                     start=(kt == 0), stop=(kt == ET - 1))
                qT = qkp.tile([128, Q], f32r, tag="qT", name=f"qT_{dt}")
                nc.vector.tensor_scalar(out=qT[:], in0=pq[:],
                                        scalar1=bqkv_sb[:, dt:dt + 1], scalar2=None,
                                        op0=ALU.add)
                # kT [128, KV]
                kT = qkp.tile([128, KV], f32r, tag="kT", name=f"kT_{dt}")
                pk = [pmm.tile([128, 512], f32, tag="mm", name=f"pk_{dt}_{h}")
                      for h in range(2)]
                for kt in range(ET):
                    wt = wsp.tile([128, 128], f32r, tag="wk", name=f"wk_{dt}_{kt}")
                    nc.sync.dma_start(
                        out=wt[:],
                        in_=wqkvT_d[kt * 128:(kt + 1) * 128,
                                    E + dt * 128: E + (dt + 1) * 128].bitcast(f32r))
                    for h in range(2):
                        nc.tensor.matmul(pk[h][:], wt[:],
                                         lx[kt][:, h * 512:(h + 1) * 512],
                                         start=(kt == 0), stop=(kt == ET - 1))
                for h in range(2):
                    nc.vector.tensor_scalar(out=kT[:, h * 512:(h + 1) * 512],
                                            in0=pk[h][:],
                                            scalar1=bqkv_sb[:, 8 + dt:9 + dt],
                                            scalar2=None, op0=ALU.add)

                for hh in range(2):
                    hsub = slice(hh * 64, hh * 64 + 64)
                    hloc = (dt - half * 4) * 2 + hh
                    pav_t = pav.tile([65, Q], f32, tag="av", name=f"pav_{dt}_{hh}")
                    for tp in range(ET // 2):
                        psc = pmm2.tile([128, 2 * Q], f32, tag="sc",
                                        name=f"psc_{dt}_{hh}_{tp}")
                        for s_ in range(2):
                            tt = tp * 2 + s_
                            nc.tensor.matmul(psc[:, s_ * Q:(s_ + 1) * Q],
                                             kT[hsub, tt * 128:(tt + 1) * 128],
                                             qT[hsub, :],
                                             start=True, stop=True,
                                             skip_group_check=True)
                        at = attnp.tile([128, 2 * Q], f32r, tag="attn", bufs=2,
                                        name=f"attn_{dt}_{hh}_{tp}")
                        nc.scalar.activation(out=at[:], in_=psc[:], func=AF.Exp,
                                             scale=0.125)
                        for s_ in range(2):
                            tt = tp * 2 + s_
                            nc.tensor.matmul(
                                pav_t[:],
                                v_sb[tt][:].rearrange("p (h d) -> p h d", h=8)[:, hloc, :],
                                at[:, s_ * Q:(s_ + 1) * Q],
                                start=(tt == 0), stop=(tt == ET - 1))
                    # stash unnormalized o and the softmax denominator; normalize
                    # per-half below so the slow reciprocal runs once, off the
                    # per-head critical path
                    nc.vector.tensor_copy(out=oT[dt][hsub, :], in_=pav_t[0:64, :])
                    dtmp = attnp.tile([1, Q], f32, tag="dtmp", name=f"dtmp_{dt}_{hh}",
                                      bufs=2)
                    nc.vector.tensor_copy(out=dtmp[:], in_=pav_t[64:65, :])
                    nc.gpsimd.dma_start(out=denAll[hloc:hloc + 1, :], in_=dtmp[:])

            recipA = statp.tile([8, Q], f32r, tag="recipA", name=f"recipA_{half}",
                                bufs=2)
            with nc.allow_low_precision(reason="f32r keeps fp32 bit layout"):
                nc.vector.reciprocal(recipA[:], denAll[:])
            for dt in range(half * 4, half * 4 + 4):
                prb = pmm.tile([128, Q], f32, tag="mm", name=f"prb_{dt}")
                nc.tensor.matmul(prb[:], sel_tiles[dt % 4][:], recipA[:],
                                 start=True, stop=True)
                rB = attnp.tile([128, Q], f32, tag="rB", name=f"rB_{dt}", bufs=2)
                nc.vector.tensor_copy(out=rB[:], in_=prb[:])
                nc.vector.tensor_mul(oT[dt][:, :], oT[dt][:, :], rB[:])

        attnp.release()
        qkp.release()
        vp.release()
        wvp.release()
        lxp.release()

        # ---------- phase 3: out projection + residual -> x2T ----------
        x2 = []
        for et in range(ET):
            po = pmm.tile([128, Q], f32, tag="mm", name=f"po_{et}")
            for dt in range(ET):
                wt = wsp.tile([128, 128], f32r, tag="wo", name=f"wo_{et}_{dt}", bufs=8)
                nc.sync.dma_start(
                    out=wt[:],
                    in_=woT_d[dt * 128:(dt + 1) * 128, et * 128:(et + 1) * 128].bitcast(f32r))
                nc.tensor.matmul(po[:], wt[:], oT[dt][:],
                                 start=(dt == 0), stop=(dt == ET - 1))
            xt = outp.tile([128, Q], f32r, tag=f"x2_{et}", name=f"x2_{et}")
            nc.scalar.activation(out=xt[:], in_=po[:], func=AF.Identity,
                                 bias=bo_sb[:, et:et + 1], scale=1.0)
            nc.vector.tensor_add(xt[:], xt[:], xq[et][:])
            nc.sync.dma_start(out=x2T_d[et * 128:(et + 1) * 128, :], in_=xt[:].bitcast(f32))
            x2.append(xt)

        # ---------- phase 4: LN2 -> h2T ----------
        rstd2, beta2 = ln_stats(x2, Q, "ln2")
        aB2 = bcast_rows(rstd2, Q, "aB2")
        for et in range(ET):
            t = outp.tile([128, Q], f32, tag="h2", name=f"h2_{et}", bufs=2)
            pl = pmm.tile([128, 512], f32, tag="mm", name=f"pl2_{et}")
            nc.tensor.matmul(pl[:], ident[:], x2[et][:], start=True, stop=False)
            nc.tensor.matmul(pl[:], ones1[:], beta2[:, 0:Q], start=False, stop=True)
            nc.vector.tensor_mul(t[:], pl[:], aB2[:])
            nc.scalar.activation(out=t[:], in_=t[:], func=AF.Identity,
                                 scale=g2_sb[:, et:et + 1], bias=b2_sb[:, et:et + 1])
            nc.gpsimd.dma_start(out=h2T_d[et * 128:(et + 1) * 128, :], in_=t[:])

        # release remaining pools in LIFO order per space
        xqp.release()
        outp.release()
        otp.release()
        wsp.release()
        sqp.release()
        bcp.release()
        statp.release()
        consts.release()
        pav.release()
        pmm2.release()
        pmm.release()

    nc.compile()
    return nc


def _build_launch2():
    nc = bacc.Bacc("TRN2", target_bir_lowering=False, debug=False, num_devices=NCORES)

    toksT_d = nc.dram_tensor("toksT", [E, C], f32, kind="ExternalInput").ap()
    w1_d = nc.dram_tensor("w1", [E, F], f32, kind="ExternalInput").ap()
    w2_d = nc.dram_tensor("w2", [F, E], bf16, kind="ExternalInput").ap()
    b1_d = nc.dram_tensor("b1", [F, 1], f32, kind="ExternalInput").ap()
    b2_d = nc.dram_tensor("b2", [E, 1], f32, kind="ExternalInput").ap()
    outT_d = nc.dram_tensor("outT", [E, C], f32, kind="ExternalOutput").ap()

    with tile.TileContext(nc) as tc:
        with (
            tc.tile_pool(name="consts", bufs=1) as consts,
            tc.tile_pool(name="tok", bufs=1) as tokp,
            tc.tile_pool(name="hp", bufs=1) as hp,
            tc.tile_pool(name="ws", bufs=6) as wsp,
            tc.tile_pool(name="outs", bufs=3) as outs,
            tc.tile_pool(name="pg1", bufs=4, space="PSUM") as pg1,
            tc.tile_pool(name="pg2", bufs=4, space="PSUM") as pg2,
        ):
            b1_sb = consts.tile([128, FT], f32, tag="b1")
            nc.sync.dma_start(out=b1_sb[:], in_=b1_d.rearrange("(a p) o -> p (a o)", p=128))
            b2_sb = consts.tile([128, ET], f32, tag="b2")
            nc.sync.dma_start(out=b2_sb[:], in_=b2_d.rearrange("(a p) o -> p (a o)", p=128))

            toks = []
            for i in range(ET):
                t = tokp.tile([128, C], f32r, tag=f"t{i}", name=f"toks{i}")
                nc.sync.dma_start(out=t[:], in_=toksT_d[i * 128:(i + 1) * 128, :].bitcast(f32r))
                toks.append(t)

            hbf = []
            for ft in range(FT):
                hbf.append(hp.tile([128, C], bf16, tag=f"h{ft}", name=f"hbf{ft}"))

            # GEMM1: hT = gelu(w1.T @ toksT + b1)
            # weight blocks [128, 256] cover two ft tiles -> half the DMA count
            for ftp in range(FT // 2):
                blks = []
                for kt in range(ET):
                    wt = wsp.tile([128, 256], f32r, tag="w1", name=f"w1_{ftp}_{kt}",
                                  bufs=12)
                    eng = nc.sync if kt % 2 == 0 else nc.gpsimd
                    eng.dma_start(
                        out=wt[:],
                        in_=w1_d[kt * 128:(kt + 1) * 128,
                                 ftp * 256:(ftp + 1) * 256].bitcast(f32r))
                    blks.append(wt)
                for sub in range(2):
                    ft = ftp * 2 + sub
                    ps = [pg1.tile([128, w], f32, tag="g1", name=f"pg1_{ft}_{ci}")
                          for ci, (off, w) in enumerate(CT)]
                    for kt in range(ET):
                        wv = blks[kt][:, sub * 128:(sub + 1) * 128]
                        for ci, (off, w) in enumerate(CT):
                            nc.tensor.matmul(ps[ci][:], wv,
                                             toks[kt][:, off:off + w],
                                             start=(kt == 0), stop=(kt == ET - 1))
                    for ci, (off, w) in enumerate(CT):
                        nc.scalar.activation(out=hbf[ft][:, off:off + w], in_=ps[ci][:],
                                             func=_GELU, bias=b1_sb[:, ft:ft + 1],
                                             scale=1.0)

            # GEMM2: outT = w2.T @ hT + b2
            # weight blocks [128, 256] cover two et tiles, kept resident across
            # both et accumulations
            for etp in range(ET // 2):
                blks = []
                for ft in range(FT):
                    wt = wsp.tile([128, 256], bf16, tag="w2", name=f"w2_{etp}_{ft}",
                                  bufs=36)
                    eng = nc.sync if ft % 2 == 0 else nc.gpsimd
                    eng.dma_start(
                        out=wt[:],
                        in_=w2_d[ft * 128:(ft + 1) * 128, etp * 256:(etp + 1) * 256])
                    blks.append(wt)
                for sub in range(2):
                    et = etp * 2 + sub
                    ps = [pg2.tile([128, w], f32, tag="g2", name=f"pg2_{et}_{ci}")
                          for ci, (off, w) in enumerate(CT)]
                    for ft in range(FT):
                        wv = blks[ft][:, sub * 128:(sub + 1) * 128]
                        for ci, (off, w) in enumerate(CT):
                            nc.tensor.matmul(ps[ci][:], wv, hbf[ft][:, off:off + w],
                                             start=(ft == 0), stop=(ft == FT - 1))
                    for ci, (off, w) in enumerate(CT):
                        ot = outs.tile([128, 512], f32, tag="ot", name=f"ot_{et}_{ci}")
                        nc.vector.tensor_scalar(out=ot[:, 0:w], in0=ps[ci][:],
                                                scalar1=b2_sb[:, et:et + 1],
                                                scalar2=None, op0=ALU.add)
                        nc.gpsimd.dma_start(
                            out=outT_d[et * 128:(et + 1) * 128, off:off + w],
                            in_=ot[:, 0:w])

    nc.compile()
    return nc


def _get_programs():
    if "l1" not in _programs:
        _programs["l1"] = _build_launch1()
    if "l2" not in _programs:
        _programs["l2"] = _build_launch2()
    return _programs["l1"], _programs["l2"]


def _expert_ffn_host(toks, w1e, b1e, w2e, b2e):
    """Exact host fallback for capacity overflow (rare)."""
    from scipy.special import erf
    h = toks @ w1e + b1e
    h = 0.5 * h * (1.0 + erf(h / np.float32(np.sqrt(2.0))))
    return h.astype(np.float32) @ w2e + b2e


def kernel(**inputs):
    import ml_dtypes

    l1, l2 = _get_programs()

    x = np.ascontiguousarray(np.asarray(inputs["x"], dtype=np.float32))        # (S,B,E)
    in_w = np.asarray(inputs["in_proj_w"], dtype=np.float32)                   # (3E,E)
    in_b = np.asarray(inputs["in_proj_b"], dtype=np.float32)
    out_w = np.asarray(inputs["out_proj_w"], dtype=np.float32)
    out_b = np.asarray(inputs["out_proj_b"], dtype=np.float32)
    gate_w = np.asarray(inputs["gate_w"], dtype=np.float32)                    # (NE,E)
    w1 = np.asarray(inputs["w1"], dtype=np.float32)                            # (NE,E,F)
    b1 = np.asarray(inputs["b1"], dtype=np.float32)
    w2 = np.asarray(inputs["w2"], dtype=np.float32)                            # (NE,F,E)
    b2 = np.asarray(inputs["b2"], dtype=np.float32)
    ln1_g = np.asarray(inputs["ln1_g"], dtype=np.float32)
    ln1_b = np.asarray(inputs["ln1_b"], dtype=np.float32)
    ln2_g = np.asarray(inputs["ln2_g"], dtype=np.float32)
    ln2_b = np.asarray(inputs["ln2_b"], dtype=np.float32)

    wqkvT = np.ascontiguousarray(in_w.T)       # (E, 3E)
    woT = np.ascontiguousarray(out_w.T)        # (E, E)
    col = lambda v: np.ascontiguousarray(v.reshape(-1, 1))

    sel = np.zeros((4 * 8, 128), dtype=np.float32)
    for d4 in range(4):
        sel[d4 * 8 + 2 * d4, 0:64] = 1.0
        sel[d4 * 8 + 2 * d4 + 1, 64:128] = 1.0
    ident = np.eye(128, dtype=np.float32)

    # ---- launch 1 ----
    xT_b = [np.ascontiguousarray(x[:, b, :].T) for b in range(B)]  # (E, S) per batch
    in_maps1 = []
    for c in range(NCORES):
        b, half = divmod(c, 2)
        xb = xT_b[b]
        perm_cols = np.concatenate([
            np.arange(half * Q, half * Q + Q),
            np.arange(Q, S) if half == 0 else np.arange(0, Q),
        ])
        in_maps1.append({
            "xT": np.ascontiguousarray(xb[:, perm_cols]),
            "sel": sel,
            "ident": ident,
            "wqkvT": wqkvT, "bqkv": col(in_b),
            "woT": woT, "bo": col(out_b),
            "g1": col(ln1_g), "b1": col(ln1_b),
            "g2": col(ln2_g), "b2": col(ln2_b),
        })
    res1 = run_bass_kernel_spmd(l1, in_maps1, list(range(NCORES)))

    x2_all = np.empty((E, S, B), dtype=np.float32)
    h2_all = np.empty((E, S, B), dtype=np.float32)
    for c in range(NCORES):
        b, half = divmod(c, 2)
        sl = slice(half * Q, half * Q + Q)
        x2_all[:, sl, b] = res1.results[c]["x2T"]
        h2_all[:, sl, b] = res1.results[c]["h2T"]
    x2_flat = x2_all.reshape(E, N)      # token n = s*B + b
    h2_flat = h2_all.reshape(E, N)

    # ---- host gating: softmax over NE logits, top-2 renormalized ----
    logits = gate_w @ h2_flat                        # (NE, N)
    logits -= logits.max(axis=0, keepdims=True)
    p = np.exp(logits)
    p /= p.sum(axis=0, keepdims=True)
    ar = np.arange(N)
    i1 = np.argmax(p, axis=0)
    v1 = p[i1, ar]
    pm = p.copy()
    pm[i1, ar] = -1.0
    i2 = np.argmax(pm, axis=0)
    v2 = p[i2, ar]
    gsum = v1 + v2
    gate1 = v1 / gsum
    gate2 = v2 / gsum

    idx_list, gates_list, ov_list = [], [], []
    in_maps2 = []
    for e in range(NE):
        sel = np.where((i1 == e) | (i2 == e))[0]
        ge = np.where(i1[sel] == e, gate1[sel], gate2[sel]).astype(np.float32)
        ov = None
        if len(sel) > C:
            ov = (sel[C:], ge[C:])
            sel, ge = sel[:C], ge[:C]
        idx_list.append(sel)
        gates_list.append(ge)
        ov_list.append(ov)
        toksT = np.zeros((E, C), dtype=np.float32)
        toksT[:, :len(sel)] = h2_flat[:, sel]
        in_maps2.append({
            "toksT": toksT,
            "w1": w1[e],
            "w2": w2[e].astype(ml_dtypes.bfloat16),
            "b1": col(b1[e]),
            "b2": col(b2[e]),
        })
    res2 = run_bass_kernel_spmd(l2, in_maps2, list(range(NCORES)))

    # ---- combine ----
    out_flat = x2_flat
    for e in range(NE):
        sel, ge = idx_list[e], gates_list[e]
        out_flat[:, sel] += res2.results[e]["outT"][:, :len(sel)] * ge[None, :]
        if ov_list[e] is not None:
            osel, oge = ov_list[e]
            oo = _expert_ffn_host(h2_flat[:, osel].T, w1[e], b1[e], w2[e], b2[e])
            out_flat[:, osel] += oo.T * oge[None, :]

    return np.ascontiguousarray(
        out_flat.reshape(E, S, B).transpose(1, 2, 0)).astype(np.float32)



# revision 13
# speedup vs baseline: 1.2696x; 1.2696x over previous
"""MoE transformer layer on 8 Trainium2 NeuronCores.

Strategy:
  Launch 1 (attention): shard by (batch, head-group) -> 8 cores.
    Core (b, g) holds all 1024 tokens of batch b and computes LN1 ->
    Q/K/V for its 8 heads -> softmax -> AV -> its partial of the output
    projection, all in bf16 with features on partitions. No K/V
    duplication across cores; LN1 gain/bias are folded into the QKV
    weights on the host. Output: partial attn projection [E, S] bf16.
  Host: combine the two partials per batch + residual -> x2; LN2 ->
    h2; top-2 gating (softmax over 8 logits, renormalized); builds the
    per-expert token batches (all-to-all dispatch done on host).
  Launch 2 (expert FFN): expert-parallel, core e owns expert e.
    toksT [E, C] bf16 -> gelu(w1.T @ toks + b1) -> w2.T @ h + b2.
  Host: scatter-add combine with gate weights + residual.
"""

import numpy as np

import concourse.bass as bass
import concourse.tile as tile
from concourse import bacc, mybir
from concourse.bass_utils import run_bass_kernel_spmd

S, B, E = 1024, 4, 1024
H, DH = 16, 64
F, NE = 4096, 8
N = S * B
NCORES = 8
C = 1088         # expert capacity (max expert load for seed-0 inputs is ~1076)
CT = [(0, 512), (512, 512), (1024, 64)]  # (offset, width) token tiles in launch 2
ET = E // 128    # 8
FT = F // 128    # 32
NP = 4           # head pairs per core (8 heads)

f32 = mybir.dt.float32
f32r = mybir.dt.float32r
bf16 = mybir.dt.bfloat16
AF = mybir.ActivationFunctionType
ALU = mybir.AluOpType

_GELU = AF.Gelu

_programs = {}


def _build_launch1():
    nc = bacc.Bacc("TRN2", target_bir_lowering=False, debug=False, num_devices=NCORES)

    xT_d = nc.dram_tensor("xT", [E, S], bf16, kind="ExternalInput").ap()
    wqkv_d = nc.dram_tensor("wqkv", [E, NP * 384], bf16, kind="ExternalInput").ap()
    bqkv_d = nc.dram_tensor("bqkv", [12 * 128, 1], f32, kind="ExternalInput").ap()
    woT_d = nc.dram_tensor("woT", [512, E], bf16, kind="ExternalInput").ap()
    sel2_d = nc.dram_tensor("sel2", [2, 128], f32, kind="ExternalInput").ap()
    ident_d = nc.dram_tensor("ident", [128, 128], bf16, kind="ExternalInput").ap()
    outT_d = nc.dram_tensor("outT", [E, S], bf16, kind="ExternalOutput").ap()

    with tile.TileContext(nc) as tc:
        consts = tc.alloc_tile_pool(name="consts", bufs=1)
        statp = tc.alloc_tile_pool(name="stat", bufs=1)
        bcp = tc.alloc_tile_pool(name="bc", bufs=1)
        sqp = tc.alloc_tile_pool(name="sqp", bufs=2)
        wsp = tc.alloc_tile_pool(name="wstream", bufs=3)
        qkvp = tc.alloc_tile_pool(name="qkvp", bufs=2)
        attnp = tc.alloc_tile_pool(name="attnp", bufs=8)
        otp = tc.alloc_tile_pool(name="otp", bufs=1)
        outp = tc.alloc_tile_pool(name="outp", bufs=3)
        pmm = tc.alloc_tile_pool(name="pmm", bufs=2, space="PSUM")
        psc = tc.alloc_tile_pool(name="psc", bufs=2, space="PSUM")
        pav = tc.alloc_tile_pool(name="pav", bufs=2, space="PSUM")

        ones128 = consts.tile([128, 1], bf16, tag="ones128")
        nc.vector.memset(ones128[:], 1.0)
        ones1b = consts.tile([1, 128], bf16, tag="ones1b")
        nc.vector.memset(ones1b[:], 1.0)
        ones1f = consts.tile([1, 128], f32r, tag="ones1f")
        nc.vector.memset(ones1f[:].bitcast(f32), 1.0)
        eps = consts.tile([1, 1], f32, tag="eps")
        nc.vector.memset(eps[:], 1e-5)
        dust = consts.tile([1, 1], f32, tag="dust")

        ident = consts.tile([128, 128], bf16, tag="ident")
        nc.sync.dma_start(out=ident[:], in_=ident_d)
        sel2 = consts.tile([2, 128], f32r, tag="sel2")
        nc.sync.dma_start(out=sel2[:], in_=sel2_d.bitcast(f32r))
        bqkv_sb = consts.tile([128, 12], f32, tag="bqkv")
        nc.sync.dma_start(out=bqkv_sb[:],
                          in_=bqkv_d.rearrange("(a p) o -> p (a o)", p=128))

        # prime the ACT Ln/Exp table set while DMAs are in flight
        nc.scalar.activation(out=dust[:], in_=eps[:], func=AF.Ln, scale=1.0)
        nc.scalar.activation(out=dust[:], in_=dust[:], func=AF.Exp, scale=1.0)

        # ---------- phase 1: load x (bf16), LN1 stats ----------
        lxp = tc.alloc_tile_pool(name="lxp", bufs=1)
        xp = tc.alloc_tile_pool(name="xp", bufs=1)

        x_sb = []
        for i in range(ET):
            t = xp.tile([128, S], bf16, tag=f"x{i}", name=f"x_sb{i}")
            nc.sync.dma_start(out=t[:], in_=xT_d[i * 128:(i + 1) * 128, :])
            x_sb.append(t)

        mu = statp.tile([1, S], f32, tag="mu")
        s2 = statp.tile([1, S], f32, tag="s2")
        tmp = statp.tile([1, S], f32, tag="tmp")
        rstd = statp.tile([1, S], f32r, tag="rstd")
        betaB = statp.tile([1, S], bf16, tag="betaB")
        for h in range(2):
            cs = slice(h * 512, (h + 1) * 512)
            p1 = pmm.tile([1, 512], f32, tag="mm", name=f"st1_{h}")
            for i in range(ET):
                nc.tensor.matmul(p1[:], ones128[:], x_sb[i][:, cs],
                                 start=(i == 0), stop=(i == ET - 1))
            nc.vector.tensor_scalar(out=mu[:, cs], in0=p1[:], scalar1=1.0 / E,
                                    scalar2=None, op0=ALU.mult)
            p2 = pmm.tile([1, 512], f32, tag="mm", name=f"st2_{h}")
            for i in range(ET):
                sq = sqp.tile([128, 512], bf16, tag="sq", name=f"sq_{h}_{i}")
                nc.vector.tensor_mul(sq[:], x_sb[i][:, cs], x_sb[i][:, cs])
                nc.tensor.matmul(p2[:], ones128[:], sq[:],
                                 start=(i == 0), stop=(i == ET - 1))
            nc.vector.tensor_scalar(out=s2[:, cs], in0=p2[:], scalar1=1.0 / E,
                                    scalar2=None, op0=ALU.mult)
        nc.vector.tensor_mul(tmp[:], mu[:], mu[:])
        nc.vector.tensor_sub(s2[:], s2[:], tmp[:])
        nc.scalar.activation(out=tmp[:], in_=s2[:], func=AF.Ln, bias=eps[:], scale=1.0)
        nc.scalar.activation(out=rstd[:], in_=tmp[:], func=AF.Exp, scale=-0.5)
        nc.vector.tensor_scalar(out=betaB[:], in0=mu[:], scalar1=-1.0,
                                scalar2=None, op0=ALU.mult)

        rstdB = bcp.tile([128, S], f32, tag="rstdB")
        for h in range(2):
            cs = slice(h * 512, (h + 1) * 512)
            pb = pmm.tile([128, 512], f32, tag="mm", name=f"bcr_{h}")
            nc.tensor.matmul(pb[:], ones1f[:], rstd[:, cs],
                             start=True, stop=True)
            nc.vector.tensor_copy(out=rstdB[:, cs], in_=pb[:])

        # lx = (x - mu) * rstd   (gain/bias folded into weights host-side)
        lx = []
        for i in range(ET):
            t = lxp.tile([128, S], bf16, tag=f"lx{i}", name=f"lx{i}")
            for h in range(2):
                cs = slice(h * 512, (h + 1) * 512)
                pl = pmm.tile([128, 512], f32, tag="mm", name=f"pl_{i}_{h}")
                nc.tensor.matmul(pl[:], ident[:], x_sb[i][:, cs],
                                 start=True, stop=False)
                nc.tensor.matmul(pl[:], ones1b[:], betaB[:, cs],
                                 start=False, stop=True)
                nc.vector.tensor_mul(t[:, cs], pl[:], rstdB[:, cs])
            lx.append(t)
        xp.release()

        # out-proj weights have no deps; start their DMAs early
        wo_t = []
        for ft in range(4):
            w = wsp.tile([128, E], bf16, tag=f"wo{ft}", name=f"wo_{ft}")
            eng = nc.sync if ft % 2 == 0 else nc.gpsimd
            eng.dma_start(out=w[:], in_=woT_d[ft * 128:(ft + 1) * 128, :])
            wo_t.append(w)

        # ---------- phase 2: per head pair: QKV -> scores -> softmax -> AV ----
        oT = []      # normalized attention outputs per pair [128, S] bf16

        for p in range(NP):
            # stream this pair's qkv weights: [E, 384] = [q 128 | k 128 | v 128]
            wt = []
            for kt in range(ET):
                w = wsp.tile([128, 384], bf16, tag="wqkv", name=f"w_{p}_{kt}",
                             bufs=10)
                eng = nc.sync if kt % 2 == 0 else nc.gpsimd
                eng.dma_start(out=w[:],
                              in_=wqkv_d[kt * 128:(kt + 1) * 128,
                                         p * 384:(p + 1) * 384])
                wt.append(w)

            qT = qkvp.tile([128, S], bf16, tag="qT", name=f"qT_{p}")
            kT = qkvp.tile([128, S], bf16, tag="kT", name=f"kT_{p}")
            for h in range(2):
                cs = slice(h * 512, (h + 1) * 512)
                pq = pmm.tile([128, 512], f32, tag="mm", name=f"pq_{p}_{h}")
                for kt in range(ET):
                    nc.tensor.matmul(pq[:], wt[kt][:, 0:128], lx[kt][:, cs],
                                     start=(kt == 0), stop=(kt == ET - 1))
                nc.vector.tensor_scalar(out=qT[:, cs], in0=pq[:],
                                        scalar1=bqkv_sb[:, p:p + 1],
                                        scalar2=None, op0=ALU.add)
                pk = pmm.tile([128, 512], f32, tag="mm", name=f"pk_{p}_{h}")
                for kt in range(ET):
                    nc.tensor.matmul(pk[:], wt[kt][:, 128:256], lx[kt][:, cs],
                                     start=(kt == 0), stop=(kt == ET - 1))
                nc.vector.tensor_scalar(out=kT[:, cs], in0=pk[:],
                                        scalar1=bqkv_sb[:, 4 + p:5 + p],
                                        scalar2=None, op0=ALU.add)

            # v token-major with ones column per head: [128 tok, 130]
            v_sb = []
            for tt in range(ET):
                pv = pmm.tile([128, 128], f32, tag="mm", name=f"pv_{p}_{tt}")
                for kt in range(ET):
                    nc.tensor.matmul(pv[:], lx[kt][:, tt * 128:(tt + 1) * 128],
                                     wt[kt][:, 256:384],
                                     start=(kt == 0), stop=(kt == ET - 1))
                vt = qkvp.tile([128, 130], bf16, tag=f"v{tt}", name=f"v_{p}_{tt}")
                nc.vector.tensor_copy(
                    out=vt[:].rearrange("p (h d) -> p h d", h=2)[:, :, 0:64],
                    in_=pv[:].rearrange("p (h d) -> p h d", h=2))
                nc.vector.memset(
                    vt[:].rearrange("p (h d) -> p h d", h=2)[:, :, 64:65], 1.0)
                v_sb.append(vt)

            # scores + exp, per ktok tile; h0 rows 0-63, h1 rows 64-127 of the
            # PE array run row-tiled (base partitions 0 / 64)
            at = [[None] * ET, [None] * ET]
            for tt in range(ET):
                ps_h = [psc.tile([128, S], f32, tag="sc", name=f"sc_{p}_{tt}_{h}")
                        for h in range(2)]
                for qc in range(2):
                    for h in range(2):
                        hsub = slice(h * 64, h * 64 + 64)
                        nc.tensor.matmul(ps_h[h][:, qc * 512:(qc + 1) * 512],
                                         kT[hsub, tt * 128:(tt + 1) * 128],
                                         qT[hsub, qc * 512:(qc + 1) * 512],
                                         start=True, stop=True,
                                         skip_group_check=True)
                for h in range(2):
                    a = attnp.tile([128, S], bf16, tag="attn",
                                   name=f"at_{p}_{tt}_{h}")
                    nc.scalar.activation(out=a[:], in_=ps_h[h][:], func=AF.Exp,
                                         scale=0.125)
                    at[h][tt] = a

            # AV + denominator (ones column), then per-pair normalize
            oTp = otp.tile([128, S], f32, tag=f"oT{p}", name=f"oT_{p}")
            den_p = statp.tile([2, S], f32, tag="den", name=f"den_{p}", bufs=2)
            dtmp = [statp.tile([1, S], f32, tag=f"dt{h}", name=f"dtmp_{p}_{h}",
                               bufs=2) for h in range(2)]
            for h in range(2):
                hsub = slice(h * 64, h * 64 + 64)
                po = [pav.tile([65, 512], f32, tag="av", name=f"pav_{p}_{h}_{qc}")
                      for qc in range(2)]
                for tt in range(ET):
                    vh = v_sb[tt][:].rearrange("p (h d) -> p h d", h=2)[:, h, :]
                    for qc in range(2):
                        cs = slice(qc * 512, (qc + 1) * 512)
                        nc.tensor.matmul(po[qc][:], vh, at[h][tt][:, cs],
                                         start=(tt == 0), stop=(tt == ET - 1))
                for qc in range(2):
                    cs = slice(qc * 512, (qc + 1) * 512)
                    nc.vector.tensor_copy(out=oTp[hsub, cs], in_=po[qc][0:64, :])
                    nc.vector.tensor_copy(out=dtmp[h][:, cs], in_=po[qc][64:65, :])
                nc.gpsimd.dma_start(out=den_p[h:h + 1, :], in_=dtmp[h][:])

            recp = statp.tile([2, S], f32r, tag="recp", name=f"recp_{p}", bufs=2)
            with nc.allow_low_precision(reason="f32r keeps fp32 bit layout"):
                nc.vector.reciprocal(recp[:], den_p[:])
            ot_bf = otp.tile([128, S], bf16, tag=f"ob{p}", name=f"ob_{p}")
            for qc in range(2):
                cs = slice(qc * 512, (qc + 1) * 512)
                pr = pmm.tile([128, 512], f32, tag="mm", name=f"pr_{p}_{qc}")
                nc.tensor.matmul(pr[:], sel2[:], recp[:, cs],
                                 start=True, stop=True)
                nc.vector.tensor_mul(ot_bf[:, cs], oTp[:, cs], pr[:])
            oT.append(ot_bf)

        # ---------- phase 3: partial out projection ----------
        for et in range(ET):
            for qc in range(2):
                cs = slice(qc * 512, (qc + 1) * 512)
                po = pmm.tile([128, 512], f32, tag="mm", name=f"po_{et}_{qc}")
                for ft in range(4):
                    nc.tensor.matmul(po[:], wo_t[ft][:, et * 128:(et + 1) * 128],
                                     oT[ft][:, cs],
                                     start=(ft == 0), stop=(ft == 3))
                ot = outp.tile([128, 512], bf16, tag="out", name=f"o_{et}_{qc}")
                nc.vector.tensor_copy(out=ot[:], in_=po[:])
                eng = nc.sync if (et + qc) % 2 == 0 else nc.gpsimd
                eng.dma_start(out=outT_d[et * 128:(et + 1) * 128, cs], in_=ot[:])

        lxp.release()
        outp.release()
        otp.release()
        attnp.release()
        qkvp.release()
        wsp.release()
        sqp.release()
        bcp.release()
        statp.release()
        consts.release()
        pav.release()
        psc.release()
        pmm.release()

    nc.compile()
    return nc


def _build_launch2():
    nc = bacc.Bacc("TRN2", target_bir_lowering=False, debug=False, num_devices=NCORES)

    toksT_d = nc.dram_tensor("toksT", [E, C], bf16, kind="ExternalInput").ap()
    w1_d = nc.dram_tensor("w1", [E, F], bf16, kind="ExternalInput").ap()
    w2_d = nc.dram_tensor("w2", [F, E], bf16, kind="ExternalInput").ap()
    b1_d = nc.dram_tensor("b1", [F, 1], f32, kind="ExternalInput").ap()
    b2_d = nc.dram_tensor("b2", [E, 1], f32, kind="ExternalInput").ap()
    outT_d = nc.dram_tensor("outT", [E, C], bf16, kind="ExternalOutput").ap()

    with tile.TileContext(nc) as tc:
        with (
            tc.tile_pool(name="consts", bufs=1) as consts,
            tc.tile_pool(name="tok", bufs=1) as tokp,
            tc.tile_pool(name="hp", bufs=1) as hp,
            tc.tile_pool(name="ws", bufs=6) as wsp,
            tc.tile_pool(name="outs", bufs=3) as outs,
            tc.tile_pool(name="pg1", bufs=4, space="PSUM") as pg1,
            tc.tile_pool(name="pg2", bufs=4, space="PSUM") as pg2,
        ):
            b1_sb = consts.tile([128, FT], f32, tag="b1")
            nc.sync.dma_start(out=b1_sb[:], in_=b1_d.rearrange("(a p) o -> p (a o)", p=128))
            b2_sb = consts.tile([128, ET], f32, tag="b2")
            nc.sync.dma_start(out=b2_sb[:], in_=b2_d.rearrange("(a p) o -> p (a o)", p=128))
            dust = consts.tile([1, 1], f32, tag="dust")
            nc.scalar.activation(out=dust[:], in_=b1_sb[0:1, 0:1], func=_GELU,
                                 scale=1.0)

            toks = []
            for i in range(ET):
                t = tokp.tile([128, C], bf16, tag=f"t{i}", name=f"toks{i}")
                nc.sync.dma_start(out=t[:], in_=toksT_d[i * 128:(i + 1) * 128, :])
                toks.append(t)

            hbf = []
            for ft in range(FT):
                hbf.append(hp.tile([128, C], bf16, tag=f"h{ft}", name=f"hbf{ft}"))

            # GEMM1: hT = gelu(w1.T @ toksT + b1)
            # weight blocks [128, 512] cover four ft tiles
            for ftp in range(FT // 4):
                blks = []
                for kt in range(ET):
                    wt = wsp.tile([128, 512], bf16, tag="w1", name=f"w1_{ftp}_{kt}",
                                  bufs=18)
                    eng = nc.sync if kt % 2 == 0 else nc.gpsimd
                    eng.dma_start(
                        out=wt[:],
                        in_=w1_d[kt * 128:(kt + 1) * 128,
                                 ftp * 512:(ftp + 1) * 512])
                    blks.append(wt)
                for sub in range(4):
                    ft = ftp * 4 + sub
                    ps = [pg1.tile([128, w], f32, tag="g1", name=f"pg1_{ft}_{ci}")
                          for ci, (off, w) in enumerate(CT)]
                    for kt in range(ET):
                        wv = blks[kt][:, sub * 128:(sub + 1) * 128]
                        for ci, (off, w) in enumerate(CT):
                            nc.tensor.matmul(ps[ci][:], wv,
                                             toks[kt][:, off:off + w],
                                             start=(kt == 0), stop=(kt == ET - 1))
                    for ci, (off, w) in enumerate(CT):
                        nc.scalar.activation(out=hbf[ft][:, off:off + w], in_=ps[ci][:],
                                             func=_GELU, bias=b1_sb[:, ft:ft + 1],
                                             scale=1.0)

            # GEMM2: outT = w2.T @ hT + b2
            # weight blocks [128, 512] cover four et tiles, kept resident across
            # the et accumulations
            for etp in range(ET // 4):
                blks = []
                for ft in range(FT):
                    wt = wsp.tile([128, 512], bf16, tag="w2", name=f"w2_{etp}_{ft}",
                                  bufs=40)
                    eng = nc.sync if ft % 2 == 0 else nc.gpsimd
                    eng.dma_start(
                        out=wt[:],
                        in_=w2_d[ft * 128:(ft + 1) * 128, etp * 512:(etp + 1) * 512])
                    blks.append(wt)
                for sub in range(4):
                    et = etp * 4 + sub
                    ps = [pg2.tile([128, w], f32, tag="g2", name=f"pg2_{et}_{ci}")
                          for ci, (off, w) in enumerate(CT)]
                    for ft in range(FT):
                        wv = blks[ft][:, sub * 128:(sub + 1) * 128]
                        for ci, (off, w) in enumerate(CT):
                            nc.tensor.matmul(ps[ci][:], wv, hbf[ft][:, off:off + w],
                                             start=(ft == 0), stop=(ft == FT - 1))
                    for ci, (off, w) in enumerate(CT):
                        ot = outs.tile([128, 512], bf16, tag="ot", name=f"ot_{et}_{ci}")
                        nc.vector.tensor_scalar(out=ot[:, 0:w], in0=ps[ci][:],
                                                scalar1=b2_sb[:, et:et + 1],
                                                scalar2=None, op0=ALU.add)
                        nc.gpsimd.dma_start(
                            out=outT_d[et * 128:(et + 1) * 128, off:off + w],
                            in_=ot[:, 0:w])

    nc.compile()
    return nc


def _get_programs():
    if "l1" not in _programs:
        _programs["l1"] = _build_launch1()
    if "l2" not in _programs:
        _programs["l2"] = _build_launch2()
    return _programs["l1"], _programs["l2"]


def _expert_ffn_host(toks, w1e, b1e, w2e, b2e):
    """Exact host fallback for capacity overflow (rare)."""
    from scipy.special import erf
    h = toks @ w1e + b1e
    h = 0.5 * h * (1.0 + erf(h / np.float32(np.sqrt(2.0))))
    return h.astype(np.float32) @ w2e + b2e


def kernel(**inputs):
    import ml_dtypes

    l1, l2 = _get_programs()

    x = np.ascontiguousarray(np.asarray(inputs["x"], dtype=np.float32))        # (S,B,E)
    in_w = np.asarray(inputs["in_proj_w"], dtype=np.float32)                   # (3E,E)
    in_b = np.asarray(inputs["in_proj_b"], dtype=np.float32)
    out_w = np.asarray(inputs["out_proj_w"], dtype=np.float32)                 # (E,E)
    out_b = np.asarray(inputs["out_proj_b"], dtype=np.float32)
    gate_w = np.asarray(inputs["gate_w"], dtype=np.float32)                    # (NE,E)
    w1 = np.asarray(inputs["w1"], dtype=np.float32)                            # (NE,E,F)
    b1 = np.asarray(inputs["b1"], dtype=np.float32)
    w2 = np.asarray(inputs["w2"], dtype=np.float32)                            # (NE,F,E)
    b2 = np.asarray(inputs["b2"], dtype=np.float32)
    ln1_g = np.asarray(inputs["ln1_g"], dtype=np.float32)
    ln1_b = np.asarray(inputs["ln1_b"], dtype=np.float32)
    ln2_g = np.asarray(inputs["ln2_g"], dtype=np.float32)
    ln2_b = np.asarray(inputs["ln2_b"], dtype=np.float32)

    bf = ml_dtypes.bfloat16
    col = lambda v: np.ascontiguousarray(v.reshape(-1, 1))

    # fold LN1 gain into the QKV weights; LN1 bias into the QKV bias
    in_w_f = in_w * ln1_g[None, :]                 # (3E, E)
    in_b_f = in_b + in_w @ ln1_b                   # (3E,)

    # per-core head-group weight packs: pair-major [q 128 | k 128 | v 128]
    # head-group g covers heads 8g..8g+7 -> feature rows 512g..512(g+1)
    wqkv_g, bqkv_g, woT_g = [], [], []
    for g in range(2):
        fs = slice(512 * g, 512 * (g + 1))
        wq = in_w_f[0 * E:1 * E][fs]               # (512, E)
        wk = in_w_f[1 * E:2 * E][fs]
        wv = in_w_f[2 * E:3 * E][fs]
        bq_ = in_b_f[0 * E:1 * E][fs]
        bk_ = in_b_f[1 * E:2 * E][fs]
        pack = np.empty((E, NP * 384), dtype=np.float32)
        bias = np.zeros((12 * 128, 1), dtype=np.float32)
        for p in range(NP):
            ps = slice(128 * p, 128 * (p + 1))
            pack[:, p * 384 + 0:p * 384 + 128] = wq[ps].T
            pack[:, p * 384 + 128:p * 384 + 256] = wk[ps].T
            pack[:, p * 384 + 256:p * 384 + 384] = wv[ps].T
            bias[p * 128:(p + 1) * 128, 0] = bq_[ps]
            bias[(4 + p) * 128:(5 + p) * 128, 0] = bk_[ps]
        wqkv_g.append(np.ascontiguousarray(pack.astype(bf)))
        bqkv_g.append(bias)
        woT_g.append(np.ascontiguousarray(out_w[:, fs].T.astype(bf)))  # (512, E)

    sel2 = np.zeros((2, 128), dtype=np.float32)
    sel2[0, 0:64] = 1.0
    sel2[1, 64:128] = 1.0
    ident = np.eye(128, dtype=np.float32).astype(bf)

    # v-bias folds into a constant added on the host:
    # (o/den + bv_g) @ Wo_g.T summed over g  ->  + bv @ Wo.T (+ out_b)
    bv = in_b_f[2 * E:3 * E]
    const_out = out_b + bv @ out_w.T               # (E,)

    # ---- launch 1 ----
    xT_b = [np.ascontiguousarray(x[:, b, :].T.astype(bf)) for b in range(B)]
    in_maps1 = []
    for c in range(NCORES):
        b, g = divmod(c, 2)
        in_maps1.append({
            "xT": xT_b[b],
            "wqkv": wqkv_g[g],
            "bqkv": bqkv_g[g],
            "woT": woT_g[g],
            "sel2": sel2,
            "ident": ident,
        })
    res1 = run_bass_kernel_spmd(l1, in_maps1, list(range(NCORES)))

    # combine partials + residual -> x2 [E, S, B]; then LN2 + gating on host
    x2_all = np.empty((E, S, B), dtype=np.float32)
    for b in range(B):
        x2_all[:, :, b] = (x[:, b, :].T
                           + res1.results[2 * b]["outT"].astype(np.float32)
                           + res1.results[2 * b + 1]["outT"].astype(np.float32)
                           + const_out[:, None])
    x2_flat = x2_all.reshape(E, N)                 # token n = s*B + b

    mu = x2_flat.mean(axis=0)
    var = x2_flat.var(axis=0)
    h2_flat = ((x2_flat - mu) / np.sqrt(var + 1e-5)) * ln2_g[:, None] + ln2_b[:, None]
    h2_flat = h2_flat.astype(np.float32)

    # ---- host gating: softmax over NE logits, top-2 renormalized ----
    logits = gate_w @ h2_flat                      # (NE, N)
    logits -= logits.max(axis=0, keepdims=True)
    p = np.exp(logits)
    p /= p.sum(axis=0, keepdims=True)
    ar = np.arange(N)
    i1 = np.argmax(p, axis=0)
    v1 = p[i1, ar]
    pm = p.copy()
    pm[i1, ar] = -1.0
    i2 = np.argmax(pm, axis=0)
    v2 = p[i2, ar]
    gsum = v1 + v2
    gate1 = v1 / gsum
    gate2 = v2 / gsum

    h2_bf = h2_flat.astype(bf)
    idx_list, gates_list, ov_list = [], [], []
    in_maps2 = []
    for e in range(NE):
        sel = np.where((i1 == e) | (i2 == e))[0]
        ge = np.where(i1[sel] == e, gate1[sel], gate2[sel]).astype(np.float32)
        ov = None
        if len(sel) > C:
            ov = (sel[C:], ge[C:])
            sel, ge = sel[:C], ge[:C]
        idx_list.append(sel)
        gates_list.append(ge)
        ov_list.append(ov)
        toksT = np.zeros((E, C), dtype=bf)
        toksT[:, :len(sel)] = h2_bf[:, sel]
        in_maps2.append({
            "toksT": toksT,
            "w1": w1[e].astype(bf),
            "w2": w2[e].astype(bf),
            "b1": col(b1[e]),
            "b2": col(b2[e]),
        })
    res2 = run_bass_kernel_spmd(l2, in_maps2, list(range(NCORES)))

    # ---- combine ----
    out_flat = x2_flat
    for e in range(NE):
        sel, ge = idx_list[e], gates_list[e]
        out_flat[:, sel] += (res2.results[e]["outT"][:, :len(sel)].astype(np.float32)
                             * ge[None, :])
        if ov_list[e] is not None:
            osel, oge = ov_list[e]
            oo = _expert_ffn_host(h2_flat[:, osel].T, w1[e], b1[e], w2[e], b2[e])
            out_flat[:, osel] += oo.T * oge[None, :]

    return np.ascontiguousarray(
        out_flat.reshape(E, S, B).transpose(1, 2, 0)).astype(np.float32)


# revision 17
# speedup vs baseline: 1.3559x; 1.0680x over previous
"""MoE transformer layer on 8 Trainium2 NeuronCores.

Strategy:
  Launch 1 (attention): shard by (batch, head-group) -> 8 cores.
    Core (b, g) holds all 1024 tokens of batch b and computes LN1 ->
    Q/K/V for its 8 heads -> softmax -> AV -> its partial of the output
    projection, all in bf16 with features on partitions. No K/V
    duplication across cores; LN1 gain/bias are folded into the QKV
    weights on the host. Output: partial attn projection [E, S] bf16.
  Host: combine the two partials per batch + residual -> x2; LN2 ->
    h2; top-2 gating (softmax over 8 logits, renormalized); builds the
    per-expert token batches (all-to-all dispatch done on host).
  Launch 2 (expert FFN): expert-parallel, core e owns expert e.
    toksT [E, C] bf16 -> gelu(w1.T @ toks + b1) -> w2.T @ h + b2.
  Host: scatter-add combine with gate weights + residual.
"""

import numpy as np

import concourse.bass as bass
import concourse.tile as tile
from concourse import bacc, mybir
from concourse.bass_utils import run_bass_kernel_spmd

S, B, E = 1024, 4, 1024
H, DH = 16, 64
F, NE = 4096, 8
N = S * B
NCORES = 8
C = 1088         # expert capacity (max expert load for seed-0 inputs is ~1076)
CT = [(0, 512), (512, 512), (1024, 64)]  # (offset, width) token tiles in launch 2
ET = E // 128    # 8
FT = F // 128    # 32
NP = 4           # head pairs per core (8 heads)

f32 = mybir.dt.float32
f32r = mybir.dt.float32r
bf16 = mybir.dt.bfloat16
AF = mybir.ActivationFunctionType
ALU = mybir.AluOpType

_GELU = AF.Gelu

_programs = {}


def _build_launch1():
    nc = bacc.Bacc("TRN2", target_bir_lowering=False, debug=False, num_devices=NCORES)

    xT_d = nc.dram_tensor("xT", [E, S], bf16, kind="ExternalInput").ap()
    wqkv_d = nc.dram_tensor("wqkv", [E, NP * 384], bf16, kind="ExternalInput").ap()
    bqkv_d = nc.dram_tensor("bqkv", [12 * 128, 1], f32, kind="ExternalInput").ap()
    woT_d = nc.dram_tensor("woT", [512, E], bf16, kind="ExternalInput").ap()
    sel2_d = nc.dram_tensor("sel2", [2, 128], f32, kind="ExternalInput").ap()
    ident_d = nc.dram_tensor("ident", [128, 128], bf16, kind="ExternalInput").ap()
    outT_d = nc.dram_tensor("outT", [E, S], bf16, kind="ExternalOutput").ap()

    with tile.TileContext(nc) as tc:
        consts = tc.alloc_tile_pool(name="consts", bufs=1)
        statp = tc.alloc_tile_pool(name="stat", bufs=1)
        bcp = tc.alloc_tile_pool(name="bc", bufs=1)
        sqp = tc.alloc_tile_pool(name="sqp", bufs=2)
        wsp = tc.alloc_tile_pool(name="wstream", bufs=3)
        qkvp = tc.alloc_tile_pool(name="qkvp", bufs=2)
        attnp = tc.alloc_tile_pool(name="attnp", bufs=12)
        otp = tc.alloc_tile_pool(name="otp", bufs=1)
        outp = tc.alloc_tile_pool(name="outp", bufs=3)
        pmm = tc.alloc_tile_pool(name="pmm", bufs=2, space="PSUM")
        psc = tc.alloc_tile_pool(name="psc", bufs=2, space="PSUM")
        pav = tc.alloc_tile_pool(name="pav", bufs=2, space="PSUM")

        ones128 = consts.tile([128, 1], bf16, tag="ones128")
        nc.vector.memset(ones128[:], 1.0)
        ones1b = consts.tile([1, 128], bf16, tag="ones1b")
        nc.vector.memset(ones1b[:], 1.0)
        ones1f = consts.tile([1, 128], f32r, tag="ones1f")
        nc.vector.memset(ones1f[:].bitcast(f32), 1.0)
        eps = consts.tile([1, 1], f32, tag="eps")
        nc.vector.memset(eps[:], 1e-5)
        dust = consts.tile([1, 1], f32, tag="dust")

        ident = consts.tile([128, 128], bf16, tag="ident")
        nc.sync.dma_start(out=ident[:], in_=ident_d)
        sel_h = []
        for h in range(2):
            st = consts.tile([1, 128], f32r, tag=f"sel{h}")
            nc.sync.dma_start(out=st[:], in_=sel2_d[h:h + 1, :].bitcast(f32r))
            sel_h.append(st)
        bqkv_sb = consts.tile([128, 12], f32, tag="bqkv")
        nc.sync.dma_start(out=bqkv_sb[:],
                          in_=bqkv_d.rearrange("(a p) o -> p (a o)", p=128))

        # prime the ACT Ln/Exp table set while DMAs are in flight
        nc.scalar.activation(out=dust[:], in_=eps[:], func=AF.Ln, scale=1.0)
        nc.scalar.activation(out=dust[:], in_=dust[:], func=AF.Exp, scale=1.0)

        # ---------- phase 1: load x (bf16), LN1 stats ----------
        lxp = tc.alloc_tile_pool(name="lxp", bufs=1)
        xp = tc.alloc_tile_pool(name="xp", bufs=1)

        x_sb = []
        for i in range(ET):
            t = xp.tile([128, S], bf16, tag=f"x{i}", name=f"x_sb{i}")
            nc.sync.dma_start(out=t[:], in_=xT_d[i * 128:(i + 1) * 128, :])
            x_sb.append(t)

        mu = statp.tile([1, S], f32, tag="mu")
        s2 = statp.tile([1, S], f32, tag="s2")
        tmp = statp.tile([1, S], f32, tag="tmp")
        rstd = statp.tile([1, S], f32r, tag="rstd")
        betaB = statp.tile([1, S], bf16, tag="betaB")
        for h in range(2):
            cs = slice(h * 512, (h + 1) * 512)
            p1 = pmm.tile([1, 512], f32, tag="mm", name=f"st1_{h}")
            for i in range(ET):
                nc.tensor.matmul(p1[:], ones128[:], x_sb[i][:, cs],
                                 start=(i == 0), stop=(i == ET - 1))
            nc.vector.tensor_scalar(out=mu[:, cs], in0=p1[:], scalar1=1.0 / E,
                                    scalar2=None, op0=ALU.mult)
            p2 = pmm.tile([1, 512], f32, tag="mm", name=f"st2_{h}")
            for i in range(ET):
                sq = sqp.tile([128, 512], bf16, tag="sq", name=f"sq_{h}_{i}")
                nc.vector.tensor_mul(sq[:], x_sb[i][:, cs], x_sb[i][:, cs])
                nc.tensor.matmul(p2[:], ones128[:], sq[:],
                                 start=(i == 0), stop=(i == ET - 1))
            nc.vector.tensor_scalar(out=s2[:, cs], in0=p2[:], scalar1=1.0 / E,
                                    scalar2=None, op0=ALU.mult)
        nc.vector.tensor_mul(tmp[:], mu[:], mu[:])
        nc.vector.tensor_sub(s2[:], s2[:], tmp[:])
        nc.scalar.activation(out=tmp[:], in_=s2[:], func=AF.Ln, bias=eps[:], scale=1.0)
        nc.scalar.activation(out=rstd[:], in_=tmp[:], func=AF.Exp, scale=-0.5)
        nc.vector.tensor_scalar(out=betaB[:], in0=mu[:], scalar1=-1.0,
                                scalar2=None, op0=ALU.mult)

        rstdB = bcp.tile([128, S], f32, tag="rstdB")
        for h in range(2):
            cs = slice(h * 512, (h + 1) * 512)
            pb = pmm.tile([128, 512], f32, tag="mm", name=f"bcr_{h}")
            nc.tensor.matmul(pb[:], ones1f[:], rstd[:, cs],
                             start=True, stop=True)
            nc.vector.tensor_copy(out=rstdB[:, cs], in_=pb[:])

        # lx = (x - mu) * rstd   (gain/bias folded into weights host-side)
        lx = []
        for i in range(ET):
            t = lxp.tile([128, S], bf16, tag=f"lx{i}", name=f"lx{i}")
            for h in range(2):
                cs = slice(h * 512, (h + 1) * 512)
                pl = pmm.tile([128, 512], f32, tag="mm", name=f"pl_{i}_{h}")
                nc.tensor.matmul(pl[:], ident[:], x_sb[i][:, cs],
                                 start=True, stop=False)
                nc.tensor.matmul(pl[:], ones1b[:], betaB[:, cs],
                                 start=False, stop=True)
                nc.vector.tensor_mul(t[:, cs], pl[:], rstdB[:, cs])
            lx.append(t)
        xp.release()

        # out-proj weights have no deps; start their DMAs early
        wo_t = []
        for ft in range(4):
            w = wsp.tile([128, E], bf16, tag=f"wo{ft}", name=f"wo_{ft}")
            eng = nc.sync if ft % 2 == 0 else nc.gpsimd
            eng.dma_start(out=w[:], in_=woT_d[ft * 128:(ft + 1) * 128, :])
            wo_t.append(w)

        # ---------- phase 2: per head pair: QKV -> scores -> softmax -> AV ----
        oT = []      # normalized attention outputs per pair [128, S] bf16

        for p in range(NP):
            # stream this pair's qkv weights: [E, 384] = [q 128 | k 128 | v 128]
            wt = []
            for kt in range(ET):
                w = wsp.tile([128, 384], bf16, tag="wqkv", name=f"w_{p}_{kt}",
                             bufs=10)
                eng = nc.sync if kt % 2 == 0 else nc.gpsimd
                eng.dma_start(out=w[:],
                              in_=wqkv_d[kt * 128:(kt + 1) * 128,
                                         p * 384:(p + 1) * 384])
                wt.append(w)

            qT = qkvp.tile([128, S], bf16, tag="qT", name=f"qT_{p}")
            kT = qkvp.tile([128, S], bf16, tag="kT", name=f"kT_{p}")
            for h in range(2):
                cs = slice(h * 512, (h + 1) * 512)
                pq = pmm.tile([128, 512], f32, tag="mm", name=f"pq_{p}_{h}")
                for kt in range(ET):
                    nc.tensor.matmul(pq[:], wt[kt][:, 0:128], lx[kt][:, cs],
                                     start=(kt == 0), stop=(kt == ET - 1))
                nc.vector.tensor_scalar(out=qT[:, cs], in0=pq[:],
                                        scalar1=bqkv_sb[:, p:p + 1],
                                        scalar2=None, op0=ALU.add)
                pk = pmm.tile([128, 512], f32, tag="mm", name=f"pk_{p}_{h}")
                for kt in range(ET):
                    nc.tensor.matmul(pk[:], wt[kt][:, 128:256], lx[kt][:, cs],
                                     start=(kt == 0), stop=(kt == ET - 1))
                nc.vector.tensor_scalar(out=kT[:, cs], in0=pk[:],
                                        scalar1=bqkv_sb[:, 4 + p:5 + p],
                                        scalar2=None, op0=ALU.add)

            # v token-major with ones column per head: [128 tok, 130]
            v_sb = []
            for tt in range(ET):
                pv = pmm.tile([128, 128], f32, tag="mm", name=f"pv_{p}_{tt}")
                for kt in range(ET):
                    nc.tensor.matmul(pv[:], lx[kt][:, tt * 128:(tt + 1) * 128],
                                     wt[kt][:, 256:384],
                                     start=(kt == 0), stop=(kt == ET - 1))
                vt = qkvp.tile([128, 130], bf16, tag=f"v{tt}", name=f"v_{p}_{tt}")
                nc.vector.tensor_copy(
                    out=vt[:].rearrange("p (h d) -> p h d", h=2)[:, :, 0:64],
                    in_=pv[:].rearrange("p (h d) -> p h d", h=2))
                nc.vector.memset(
                    vt[:].rearrange("p (h d) -> p h d", h=2)[:, :, 64:65], 1.0)
                v_sb.append(vt)

            # scores + exp, per ktok tile; h0 rows 0-63, h1 rows 64-127 of the
            # PE array run row-tiled (base partitions 0 / 64)
            at = [[None] * ET, [None] * ET]
            for tt in range(ET):
                ps_h = [psc.tile([128, S], f32, tag="sc", name=f"sc_{p}_{tt}_{h}")
                        for h in range(2)]
                for qc in range(2):
                    for h in range(2):
                        hsub = slice(h * 64, h * 64 + 64)
                        nc.tensor.matmul(ps_h[h][:, qc * 512:(qc + 1) * 512],
                                         kT[hsub, tt * 128:(tt + 1) * 128],
                                         qT[hsub, qc * 512:(qc + 1) * 512],
                                         start=True, stop=True,
                                         skip_group_check=True)
                for h in range(2):
                    a = attnp.tile([128, S], bf16, tag="attn",
                                   name=f"at_{p}_{tt}_{h}")
                    nc.scalar.activation(out=a[:], in_=ps_h[h][:], func=AF.Exp,
                                         scale=0.125)
                    at[h][tt] = a

            # AV + denominator (ones column), then per-pair normalize
            oTp = otp.tile([128, S], f32, tag=f"oT{p}", name=f"oT_{p}")
            recp = [statp.tile([1, S], f32r, tag=f"recp{h}", name=f"recp_{p}_{h}",
                               bufs=2) for h in range(2)]
            for h in range(2):
                hsub = slice(h * 64, h * 64 + 64)
                den_h = statp.tile([1, S], f32, tag=f"den{h}", name=f"den_{p}_{h}",
                                   bufs=2)
                po = [pav.tile([65, 512], f32, tag="av", name=f"pav_{p}_{h}_{qc}")
                      for qc in range(2)]
                for tt in range(ET):
                    vh = v_sb[tt][:].rearrange("p (h d) -> p h d", h=2)[:, h, :]
                    for qc in range(2):
                        cs = slice(qc * 512, (qc + 1) * 512)
                        nc.tensor.matmul(po[qc][:], vh, at[h][tt][:, cs],
                                         start=(tt == 0), stop=(tt == ET - 1))
                for qc in range(2):
                    cs = slice(qc * 512, (qc + 1) * 512)
                    nc.vector.tensor_copy(out=den_h[:, cs], in_=po[qc][64:65, :])
                for qc in range(2):
                    cs = slice(qc * 512, (qc + 1) * 512)
                    nc.vector.tensor_copy(out=oTp[hsub, cs], in_=po[qc][0:64, :])
                with nc.allow_low_precision(reason="f32r keeps fp32 bit layout"):
                    nc.vector.reciprocal(recp[h][:], den_h[:])

            ot_bf = otp.tile([128, S], bf16, tag=f"ob{p}", name=f"ob_{p}")
            for qc in range(2):
                cs = slice(qc * 512, (qc + 1) * 512)
                pr = pav.tile([128, 512], f32, tag="av", name=f"pr_{p}_{qc}")
                nc.tensor.matmul(pr[:], sel_h[0][:], recp[0][:, cs],
                                 start=True, stop=False)
                nc.tensor.matmul(pr[:], sel_h[1][:], recp[1][:, cs],
                                 start=False, stop=True)
                nc.vector.tensor_mul(ot_bf[:, cs], oTp[:, cs], pr[:])
            oT.append(ot_bf)

        # ---------- phase 3: partial out projection ----------
        for et in range(ET):
            for qc in range(2):
                cs = slice(qc * 512, (qc + 1) * 512)
                po = pmm.tile([128, 512], f32, tag="mm", name=f"po_{et}_{qc}")
                for ft in range(4):
                    nc.tensor.matmul(po[:], wo_t[ft][:, et * 128:(et + 1) * 128],
                                     oT[ft][:, cs],
                                     start=(ft == 0), stop=(ft == 3))
                ot = outp.tile([128, 512], bf16, tag="out", name=f"o_{et}_{qc}")
                if et % 2 == 0:
                    nc.vector.tensor_copy(out=ot[:], in_=po[:])
                else:
                    nc.scalar.activation(out=ot[:], in_=po[:], func=AF.Identity,
                                         scale=1.0)
                eng = nc.sync if (et + qc) % 2 == 0 else nc.gpsimd
                eng.dma_start(out=outT_d[et * 128:(et + 1) * 128, cs], in_=ot[:])

        lxp.release()
        outp.release()
        otp.release()
        attnp.release()
        qkvp.release()
        wsp.release()
        sqp.release()
        bcp.release()
        statp.release()
        consts.release()
        pav.release()
        psc.release()
        pmm.release()

    nc.compile()
    return nc


def _build_launch2():
    nc = bacc.Bacc("TRN2", target_bir_lowering=False, debug=False, num_devices=NCORES)

    toksT_d = nc.dram_tensor("toksT", [E, C], bf16, kind="ExternalInput").ap()
    w1_d = nc.dram_tensor("w1", [E, F], bf16, kind="ExternalInput").ap()
    w2_d = nc.dram_tensor("w2", [F, E], bf16, kind="ExternalInput").ap()
    b1_d = nc.dram_tensor("b1", [F, 1], f32, kind="ExternalInput").ap()
    b2_d = nc.dram_tensor("b2", [E, 1], f32, kind="ExternalInput").ap()
    outT_d = nc.dram_tensor("outT", [E, C], bf16, kind="ExternalOutput").ap()

    with tile.TileContext(nc) as tc:
        with (
            tc.tile_pool(name="consts", bufs=1) as consts,
            tc.tile_pool(name="tok", bufs=1) as tokp,
            tc.tile_pool(name="hp", bufs=1) as hp,
            tc.tile_pool(name="ws", bufs=6) as wsp,
            tc.tile_pool(name="outs", bufs=3) as outs,
            tc.tile_pool(name="pg1", bufs=4, space="PSUM") as pg1,
            tc.tile_pool(name="pg2", bufs=4, space="PSUM") as pg2,
        ):
            b1_sb = consts.tile([128, FT], f32, tag="b1")
            nc.sync.dma_start(out=b1_sb[:], in_=b1_d.rearrange("(a p) o -> p (a o)", p=128))
            b2_sb = consts.tile([128, ET], f32, tag="b2")
            nc.sync.dma_start(out=b2_sb[:], in_=b2_d.rearrange("(a p) o -> p (a o)", p=128))
            dust = consts.tile([1, 1], f32, tag="dust")
            nc.scalar.activation(out=dust[:], in_=b1_sb[0:1, 0:1], func=_GELU,
                                 scale=1.0)

            toks = []
            for i in range(ET):
                t = tokp.tile([128, C], bf16, tag=f"t{i}", name=f"toks{i}")
                nc.sync.dma_start(out=t[:], in_=toksT_d[i * 128:(i + 1) * 128, :])
                toks.append(t)

            hbf = []
            for ft in range(FT):
                hbf.append(hp.tile([128, C], bf16, tag=f"h{ft}", name=f"hbf{ft}"))

            # GEMM1: hT = gelu(w1.T @ toksT + b1)
            # weight blocks [128, 512] cover four ft tiles
            for ftp in range(FT // 4):
                blks = []
                for kt in range(ET):
                    wt = wsp.tile([128, 512], bf16, tag="w1", name=f"w1_{ftp}_{kt}",
                                  bufs=18)
                    eng = nc.sync if kt % 2 == 0 else nc.gpsimd
                    eng.dma_start(
                        out=wt[:],
                        in_=w1_d[kt * 128:(kt + 1) * 128,
                                 ftp * 512:(ftp + 1) * 512])
                    blks.append(wt)
                for sub in range(4):
                    ft = ftp * 4 + sub
                    ps = [pg1.tile([128, w], f32, tag="g1", name=f"pg1_{ft}_{ci}")
                          for ci, (off, w) in enumerate(CT)]
                    for kt in range(ET):
                        wv = blks[kt][:, sub * 128:(sub + 1) * 128]
                        for ci, (off, w) in enumerate(CT):
                            nc.tensor.matmul(ps[ci][:], wv,
                                             toks[kt][:, off:off + w],
                                             start=(kt == 0), stop=(kt == ET - 1))
                    for ci, (off, w) in enumerate(CT):
                        nc.scalar.activation(out=hbf[ft][:, off:off + w], in_=ps[ci][:],
                                             func=_GELU, bias=b1_sb[:, ft:ft + 1],
                                             scale=1.0)

            # GEMM2: outT = w2.T @ hT + b2
            # weight blocks [128, 512] cover four et tiles, kept resident across
            # the et accumulations
            for etp in range(ET // 4):
                blks = []
                for ft in range(FT):
                    wt = wsp.tile([128, 512], bf16, tag="w2", name=f"w2_{etp}_{ft}",
                                  bufs=40)
                    eng = nc.sync if ft % 2 == 0 else nc.gpsimd
                    eng.dma_start(
                        out=wt[:],
                        in_=w2_d[ft * 128:(ft + 1) * 128, etp * 512:(etp + 1) * 512])
                    blks.append(wt)
                for sub in range(4):
                    et = etp * 4 + sub
                    ps = [pg2.tile([128, w], f32, tag="g2", name=f"pg2_{et}_{ci}")
                          for ci, (off, w) in enumerate(CT)]
                    for ft in range(FT):
                        wv = blks[ft][:, sub * 128:(sub + 1) * 128]
                        for ci, (off, w) in enumerate(CT):
                            nc.tensor.matmul(ps[ci][:], wv, hbf[ft][:, off:off + w],
                                             start=(ft == 0), stop=(ft == FT - 1))
                    for ci, (off, w) in enumerate(CT):
                        ot = outs.tile([128, 512], bf16, tag="ot", name=f"ot_{et}_{ci}")
                        nc.vector.tensor_scalar(out=ot[:, 0:w], in0=ps[ci][:],
                                                scalar1=b2_sb[:, et:et + 1],
                                                scalar2=None, op0=ALU.add)
                        nc.gpsimd.dma_start(
                            out=outT_d[et * 128:(et + 1) * 128, off:off + w],
                            in_=ot[:, 0:w])

    nc.compile()
    return nc


def _get_programs():
    if "l1" not in _programs:
        _programs["l1"] = _build_launch1()
    if "l2" not in _programs:
        _programs["l2"] = _build_launch2()
    return _programs["l1"], _programs["l2"]


def _expert_ffn_host(toks, w1e, b1e, w2e, b2e):
    """Exact host fallback for capacity overflow (rare)."""
    from scipy.special import erf
    h = toks @ w1e + b1e
    h = 0.5 * h * (1.0 + erf(h / np.float32(np.sqrt(2.0))))
    return h.astype(np.float32) @ w2e + b2e


def kernel(**inputs):
    import ml_dtypes

    l1, l2 = _get_programs()

    x = np.ascontiguousarray(np.asarray(inputs["x"], dtype=np.float32))        # (S,B,E)
    in_w = np.asarray(inputs["in_proj_w"], dtype=np.float32)                   # (3E,E)
    in_b = np.asarray(inputs["in_proj_b"], dtype=np.float32)
    out_w = np.asarray(inputs["out_proj_w"], dtype=np.float32)                 # (E,E)
    out_b = np.asarray(inputs["out_proj_b"], dtype=np.float32)
    gate_w = np.asarray(inputs["gate_w"], dtype=np.float32)                    # (NE,E)
    w1 = np.asarray(inputs["w1"], dtype=np.float32)                            # (NE,E,F)
    b1 = np.asarray(inputs["b1"], dtype=np.float32)
    w2 = np.asarray(inputs["w2"], dtype=np.float32)                            # (NE,F,E)
    b2 = np.asarray(inputs["b2"], dtype=np.float32)
    ln1_g = np.asarray(inputs["ln1_g"], dtype=np.float32)
    ln1_b = np.asarray(inputs["ln1_b"], dtype=np.float32)
    ln2_g = np.asarray(inputs["ln2_g"], dtype=np.float32)
    ln2_b = np.asarray(inputs["ln2_b"], dtype=np.float32)

    bf = ml_dtypes.bfloat16
    col = lambda v: np.ascontiguousarray(v.reshape(-1, 1))

    # fold LN1 gain into the QKV weights; LN1 bias into the QKV bias
    in_w_f = in_w * ln1_g[None, :]                 # (3E, E)
    in_b_f = in_b + in_w @ ln1_b                   # (3E,)

    # per-core head-group weight packs: pair-major [q 128 | k 128 | v 128]
    # head-group g covers heads 8g..8g+7 -> feature rows 512g..512(g+1)
    wqkv_g, bqkv_g, woT_g = [], [], []
    for g in range(2):
        fs = slice(512 * g, 512 * (g + 1))
        wq = in_w_f[0 * E:1 * E][fs]               # (512, E)
        wk = in_w_f[1 * E:2 * E][fs]
        wv = in_w_f[2 * E:3 * E][fs]
        bq_ = in_b_f[0 * E:1 * E][fs]
        bk_ = in_b_f[1 * E:2 * E][fs]
        pack = np.empty((E, NP * 384), dtype=np.float32)
        bias = np.zeros((12 * 128, 1), dtype=np.float32)
        for p in range(NP):
            ps = slice(128 * p, 128 * (p + 1))
            pack[:, p * 384 + 0:p * 384 + 128] = wq[ps].T
            pack[:, p * 384 + 128:p * 384 + 256] = wk[ps].T
            pack[:, p * 384 + 256:p * 384 + 384] = wv[ps].T
            bias[p * 128:(p + 1) * 128, 0] = bq_[ps]
            bias[(4 + p) * 128:(5 + p) * 128, 0] = bk_[ps]
        wqkv_g.append(np.ascontiguousarray(pack.astype(bf)))
        bqkv_g.append(bias)
        woT_g.append(np.ascontiguousarray(out_w[:, fs].T.astype(bf)))  # (512, E)

    sel2 = np.zeros((2, 128), dtype=np.float32)
    sel2[0, 0:64] = 1.0
    sel2[1, 64:128] = 1.0
    ident = np.eye(128, dtype=np.float32).astype(bf)

    # v-bias folds into a constant added on the host:
    # (o/den + bv_g) @ Wo_g.T summed over g  ->  + bv @ Wo.T (+ out_b)
    bv = in_b_f[2 * E:3 * E]
    const_out = out_b + bv @ out_w.T               # (E,)

    # ---- launch 1 ----
    xT_b = [np.ascontiguousarray(x[:, b, :].T.astype(bf)) for b in range(B)]
    in_maps1 = []
    for c in range(NCORES):
        b, g = divmod(c, 2)
        in_maps1.append({
            "xT": xT_b[b],
            "wqkv": wqkv_g[g],
            "bqkv": bqkv_g[g],
            "woT": woT_g[g],
            "sel2": sel2,
            "ident": ident,
        })
    res1 = run_bass_kernel_spmd(l1, in_maps1, list(range(NCORES)))

    # combine partials + residual -> x2 [E, S, B]; then LN2 + gating on host
    x2_all = np.empty((E, S, B), dtype=np.float32)
    for b in range(B):
        x2_all[:, :, b] = (x[:, b, :].T
                           + res1.results[2 * b]["outT"].astype(np.float32)
                           + res1.results[2 * b + 1]["outT"].astype(np.float32)
                           + const_out[:, None])
    x2_flat = x2_all.reshape(E, N)                 # token n = s*B + b

    mu = x2_flat.mean(axis=0)
    var = x2_flat.var(axis=0)
    h2_flat = ((x2_flat - mu) / np.sqrt(var + 1e-5)) * ln2_g[:, None] + ln2_b[:, None]
    h2_flat = h2_flat.astype(np.float32)

    # ---- host gating: softmax over NE logits, top-2 renormalized ----
    logits = gate_w @ h2_flat                      # (NE, N)
    logits -= logits.max(axis=0, keepdims=True)
    p = np.exp(logits)
    p /= p.sum(axis=0, keepdims=True)
    ar = np.arange(N)
    i1 = np.argmax(p, axis=0)
    v1 = p[i1, ar]
    pm = p.copy()
    pm[i1, ar] = -1.0
    i2 = np.argmax(pm, axis=0)
    v2 = p[i2, ar]
    gsum = v1 + v2
    gate1 = v1 / gsum
    gate2 = v2 / gsum

    h2_bf = h2_flat.astype(bf)
    idx_list, gates_list, ov_list = [], [], []
    in_maps2 = []
    for e in range(NE):
        sel = np.where((i1 == e) | (i2 == e))[0]
        ge = np.where(i1[sel] == e, gate1[sel], gate2[sel]).astype(np.float32)
        ov = None
        if len(sel) > C:
            ov = (sel[C:], ge[C:])
            sel, ge = sel[:C], ge[:C]
        idx_list.append(sel)
        gates_list.append(ge)
        ov_list.append(ov)
        toksT = np.zeros((E, C), dtype=bf)
        toksT[:, :len(sel)] = h2_bf[:, sel]
        in_maps2.append({
            "toksT": toksT,
            "w1": w1[e].astype(bf),
            "w2": w2[e].astype(bf),
            "b1": col(b1[e]),
            "b2": col(b2[e]),
        })
    res2 = run_bass_kernel_spmd(l2, in_maps2, list(range(NCORES)))

    # ---- combine ----
    out_flat = x2_flat
    for e in range(NE):
        sel, ge = idx_list[e], gates_list[e]
        out_flat[:, sel] += (res2.results[e]["outT"][:, :len(sel)].astype(np.float32)
                             * ge[None, :])
        if ov_list[e] is not None:
            osel, oge = ov_list[e]
            oo = _expert_ffn_host(h2_flat[:, osel].T, w1[e], b1[e], w2[e], b2[e])
            out_flat[:, osel] += oo.T * oge[None, :]

    return np.ascontiguousarray(
        out_flat.reshape(E, S, B).transpose(1, 2, 0)).astype(np.float32)


# revision 25
# speedup vs baseline: 1.4012x; 1.0334x over previous
"""MoE transformer layer on 8 Trainium2 NeuronCores.

Strategy:
  Launch 1 (attention): shard by (batch, head-group) -> 8 cores.
    Core (b, g) holds all 1024 tokens of batch b and computes LN1 ->
    Q/K/V for its 8 heads -> softmax -> AV -> its partial of the output
    projection, all in bf16 with features on partitions. No K/V
    duplication across cores; LN1 gain/bias are folded into the QKV
    weights on the host. Output: partial attn projection [E, S] bf16.
  Host: combine the two partials per batch + residual -> x2; LN2 ->
    h2; top-2 gating (softmax over 8 logits, renormalized); builds the
    per-expert token batches (all-to-all dispatch done on host).
  Launch 2 (expert FFN): expert-parallel, core e owns expert e.
    toksT [E, C] bf16 -> gelu(w1.T @ toks + b1) -> w2.T @ h + b2.
  Host: scatter-add combine with gate weights + residual.
"""

import numpy as np

import concourse.bass as bass
import concourse.tile as tile
from concourse import bacc, mybir
from concourse.bass_utils import run_bass_kernel_spmd

S, B, E = 1024, 4, 1024
H, DH = 16, 64
F, NE = 4096, 8
N = S * B
NCORES = 8
C = 1088         # expert capacity (max expert load for seed-0 inputs is ~1076)
CT = [(0, 512), (512, 512), (1024, 64)]  # (offset, width) token tiles in launch 2
ET = E // 128    # 8
FT = F // 128    # 32
NP = 4           # head pairs per core (8 heads)

f32 = mybir.dt.float32
f32r = mybir.dt.float32r
bf16 = mybir.dt.bfloat16
AF = mybir.ActivationFunctionType
ALU = mybir.AluOpType

_GELU = AF.Gelu

_programs = {}


def _build_launch1():
    nc = bacc.Bacc("TRN2", target_bir_lowering=False, debug=False, num_devices=NCORES)

    xT_d = nc.dram_tensor("xT", [E, S], bf16, kind="ExternalInput").ap()
    wqkv_d = nc.dram_tensor("wqkv", [E, NP * 384], bf16, kind="ExternalInput").ap()
    bqkv_d = nc.dram_tensor("bqkv", [12 * 128, 1], f32, kind="ExternalInput").ap()
    woT_d = nc.dram_tensor("woT", [512, E], bf16, kind="ExternalInput").ap()
    sel2_d = nc.dram_tensor("sel2", [2, 128], f32, kind="ExternalInput").ap()
    ident_d = nc.dram_tensor("ident", [128, 128], bf16, kind="ExternalInput").ap()
    outT_d = nc.dram_tensor("outT", [E, S], bf16, kind="ExternalOutput").ap()

    with tile.TileContext(nc) as tc:
        consts = tc.alloc_tile_pool(name="consts", bufs=1)
        statp = tc.alloc_tile_pool(name="stat", bufs=1)
        bcp = tc.alloc_tile_pool(name="bc", bufs=1)
        sqp = tc.alloc_tile_pool(name="sqp", bufs=2)
        wsp = tc.alloc_tile_pool(name="wstream", bufs=3)
        qkvp = tc.alloc_tile_pool(name="qkvp", bufs=2)
        attnp = tc.alloc_tile_pool(name="attnp", bufs=12)
        otp = tc.alloc_tile_pool(name="otp", bufs=1)
        outp = tc.alloc_tile_pool(name="outp", bufs=3)
        pmm = tc.alloc_tile_pool(name="pmm", bufs=2, space="PSUM")
        psc = tc.alloc_tile_pool(name="psc", bufs=2, space="PSUM")
        pav = tc.alloc_tile_pool(name="pav", bufs=2, space="PSUM")

        ones128 = consts.tile([128, 1], bf16, tag="ones128")
        nc.vector.memset(ones128[:], 1.0)
        ones1b = consts.tile([1, 128], bf16, tag="ones1b")
        nc.vector.memset(ones1b[:], 1.0)
        ones1f = consts.tile([1, 128], f32r, tag="ones1f")
        nc.vector.memset(ones1f[:].bitcast(f32), 1.0)
        eps = consts.tile([1, 1], f32, tag="eps")
        nc.vector.memset(eps[:], 1e-5)
        dust = consts.tile([1, 1], f32, tag="dust")

        ident = consts.tile([128, 128], bf16, tag="ident")
        nc.sync.dma_start(out=ident[:], in_=ident_d)
        sel_h = []
        for h in range(2):
            st = consts.tile([1, 128], f32r, tag=f"sel{h}")
            nc.sync.dma_start(out=st[:], in_=sel2_d[h:h + 1, :].bitcast(f32r))
            sel_h.append(st)
        bqkv_sb = consts.tile([128, 12], f32, tag="bqkv")
        nc.sync.dma_start(out=bqkv_sb[:],
                          in_=bqkv_d.rearrange("(a p) o -> p (a o)", p=128))

        # prime the ACT Ln/Exp table set while DMAs are in flight
        nc.scalar.activation(out=dust[:], in_=eps[:], func=AF.Ln, scale=1.0)
        nc.scalar.activation(out=dust[:], in_=dust[:], func=AF.Exp, scale=1.0)

        # ---------- phase 1: load x (bf16), LN1 stats ----------
        lxp = tc.alloc_tile_pool(name="lxp", bufs=1)
        xp = tc.alloc_tile_pool(name="xp", bufs=1)

        xbig = xp.tile([128, ET * S], bf16, tag="x", name="x_sb")
        xT_r = xT_d.rearrange("(a p) c -> p a c", p=128)
        xbig_r = xbig[:].rearrange("p (a c) -> p a c", a=ET)
        nc.sync.dma_start(out=xbig_r[:, 0:4, :], in_=xT_r[:, 0:4, :])
        nc.scalar.dma_start(out=xbig_r[:, 4:8, :], in_=xT_r[:, 4:8, :])
        x_sb = [xbig[:, i * S:(i + 1) * S] for i in range(ET)]

        mu = statp.tile([1, S], f32, tag="mu")
        s2 = statp.tile([1, S], f32, tag="s2")
        tmp = statp.tile([1, S], f32, tag="tmp")
        rstd = statp.tile([1, S], f32r, tag="rstd")
        betaB = statp.tile([1, S], bf16, tag="betaB")
        for h in range(2):
            cs = slice(h * 512, (h + 1) * 512)
            p1 = pmm.tile([1, 512], f32, tag="mm", name=f"st1_{h}")
            for i in range(ET):
                nc.tensor.matmul(p1[:], ones128[:], x_sb[i][:, cs],
                                 start=(i == 0), stop=(i == ET - 1))
            nc.vector.tensor_scalar(out=mu[:, cs], in0=p1[:], scalar1=1.0 / E,
                                    scalar2=None, op0=ALU.mult)
            p2 = pmm.tile([1, 512], f32, tag="mm", name=f"st2_{h}")
            for i in range(ET):
                sq = sqp.tile([128, 512], bf16, tag="sq", name=f"sq_{h}_{i}")
                nc.vector.tensor_mul(sq[:], x_sb[i][:, cs], x_sb[i][:, cs])
                nc.tensor.matmul(p2[:], ones128[:], sq[:],
                                 start=(i == 0), stop=(i == ET - 1))
            nc.vector.tensor_scalar(out=s2[:, cs], in0=p2[:], scalar1=1.0 / E,
                                    scalar2=None, op0=ALU.mult)
        nc.vector.tensor_mul(tmp[:], mu[:], mu[:])
        nc.vector.tensor_sub(s2[:], s2[:], tmp[:])
        nc.scalar.activation(out=tmp[:], in_=s2[:], func=AF.Ln, bias=eps[:], scale=1.0)
        nc.scalar.activation(out=rstd[:], in_=tmp[:], func=AF.Exp, scale=-0.5)
        nc.vector.tensor_scalar(out=betaB[:], in0=mu[:], scalar1=-1.0,
                                scalar2=None, op0=ALU.mult)

        rstdB = bcp.tile([128, S], f32, tag="rstdB")
        for h in range(2):
            cs = slice(h * 512, (h + 1) * 512)
            pb = pmm.tile([128, 512], f32, tag="mm", name=f"bcr_{h}")
            nc.tensor.matmul(pb[:], ones1f[:], rstd[:, cs],
                             start=True, stop=True)
            nc.vector.tensor_copy(out=rstdB[:, cs], in_=pb[:])

        # lx = (x - mu) * rstd   (gain/bias folded into weights host-side)
        lxbig = lxp.tile([128, ET * S], bf16, tag="lx", name="lx")
        lx = [lxbig[:, i * S:(i + 1) * S] for i in range(ET)]
        for i in range(ET):
            for h in range(2):
                cs = slice(h * 512, (h + 1) * 512)
                pl = pmm.tile([128, 512], f32, tag="mm", name=f"pl_{i}_{h}")
                nc.tensor.matmul(pl[:], ident[:], x_sb[i][:, cs],
                                 start=True, stop=False)
                nc.tensor.matmul(pl[:], ones1b[:], betaB[:, cs],
                                 start=False, stop=True)
                nc.vector.tensor_mul(lx[i][:, cs], pl[:], rstdB[:, cs])
        xp.release()

        # out-proj weights have no deps; start their DMA early
        wo_big = wsp.tile([128, 4 * E], bf16, tag="wo", name="wo")
        nc.gpsimd.dma_start(
            out=wo_big[:].rearrange("p (a c) -> p a c", a=4),
            in_=woT_d.rearrange("(a p) c -> p a c", p=128))
        wo_t = [wo_big[:, ft * E:(ft + 1) * E] for ft in range(4)]

        # ---------- phase 2: per head pair: QKV -> scores -> softmax -> AV ----
        oT = []      # normalized attention outputs per pair [128, S] bf16

        for p in range(NP):
            # stream this pair's qkv weights: [E, 384] = [q 128 | k 128 | v 128]
            wbig = wsp.tile([128, ET * 384], bf16, tag="wqkv", name=f"w_{p}",
                            bufs=2)
            eng = (nc.sync, nc.scalar, nc.gpsimd)[p % 3]
            eng.dma_start(
                out=wbig[:].rearrange("p (a c) -> p a c", a=ET),
                in_=wqkv_d.rearrange("(a p) c -> p a c", p=128)[:, :,
                                                               p * 384:(p + 1) * 384])
            wt = [wbig[:, kt * 384:(kt + 1) * 384] for kt in range(ET)]

            qT = qkvp.tile([128, S], bf16, tag="qT", name=f"qT_{p}")
            kT = qkvp.tile([128, S], bf16, tag="kT", name=f"kT_{p}")
            for h in range(2):
                cs = slice(h * 512, (h + 1) * 512)
                pq = pmm.tile([128, 512], f32, tag="mm", name=f"pq_{p}_{h}")
                for kt in range(ET):
                    nc.tensor.matmul(pq[:], wt[kt][:, 0:128], lx[kt][:, cs],
                                     start=(kt == 0), stop=(kt == ET - 1))
                nc.vector.tensor_scalar(out=qT[:, cs], in0=pq[:],
                                        scalar1=bqkv_sb[:, p:p + 1],
                                        scalar2=None, op0=ALU.add)
                pk = pmm.tile([128, 512], f32, tag="mm", name=f"pk_{p}_{h}")
                for kt in range(ET):
                    nc.tensor.matmul(pk[:], wt[kt][:, 128:256], lx[kt][:, cs],
                                     start=(kt == 0), stop=(kt == ET - 1))
                nc.vector.tensor_scalar(out=kT[:, cs], in0=pk[:],
                                        scalar1=bqkv_sb[:, 4 + p:5 + p],
                                        scalar2=None, op0=ALU.add)

            # v token-major with ones column per head: [128 tok, 130]
            v_sb = []
            for tt in range(ET):
                pv = pmm.tile([128, 128], f32, tag="mm", name=f"pv_{p}_{tt}")
                for kt in range(ET):
                    nc.tensor.matmul(pv[:], lx[kt][:, tt * 128:(tt + 1) * 128],
                                     wt[kt][:, 256:384],
                                     start=(kt == 0), stop=(kt == ET - 1))
                vt = qkvp.tile([128, 130], bf16, tag=f"v{tt}", name=f"v_{p}_{tt}")
                nc.vector.tensor_copy(
                    out=vt[:].rearrange("p (h d) -> p h d", h=2)[:, :, 0:64],
                    in_=pv[:].rearrange("p (h d) -> p h d", h=2))
                nc.vector.memset(
                    vt[:].rearrange("p (h d) -> p h d", h=2)[:, :, 64:65], 1.0)
                v_sb.append(vt)

            # scores + exp, per ktok tile; h0 rows 0-63, h1 rows 64-127 of the
            # PE array run row-tiled (base partitions 0 / 64)
            at = [[None] * ET, [None] * ET]
            for tt in range(ET):
                ps_h = [psc.tile([128, S], f32, tag="sc", name=f"sc_{p}_{tt}_{h}")
                        for h in range(2)]
                for qc in range(2):
                    for h in range(2):
                        hsub = slice(h * 64, h * 64 + 64)
                        nc.tensor.matmul(ps_h[h][:, qc * 512:(qc + 1) * 512],
                                         kT[hsub, tt * 128:(tt + 1) * 128],
                                         qT[hsub, qc * 512:(qc + 1) * 512],
                                         start=True, stop=True,
                                         skip_group_check=True)
                for h in range(2):
                    a = attnp.tile([128, S], bf16, tag="attn",
                                   name=f"at_{p}_{tt}_{h}")
                    nc.scalar.activation(out=a[:], in_=ps_h[h][:], func=AF.Exp,
                                         scale=0.125)
                    at[h][tt] = a

            # AV + denominator (ones column), then per-pair normalize
            oTp = otp.tile([128, S], f32, tag=f"oT{p}", name=f"oT_{p}")
            recp = [statp.tile([1, S], f32r, tag=f"recp{h}", name=f"recp_{p}_{h}",
                               bufs=2) for h in range(2)]
            for h in range(2):
                hsub = slice(h * 64, h * 64 + 64)
                den_h = statp.tile([1, S], f32, tag=f"den{h}", name=f"den_{p}_{h}",
                                   bufs=2)
                po = [pav.tile([65, 512], f32, tag="av", name=f"pav_{p}_{h}_{qc}")
                      for qc in range(2)]
                for tt in range(ET):
                    vh = v_sb[tt][:].rearrange("p (h d) -> p h d", h=2)[:, h, :]
                    for qc in range(2):
                        cs = slice(qc * 512, (qc + 1) * 512)
                        nc.tensor.matmul(po[qc][:], vh, at[h][tt][:, cs],
                                         start=(tt == 0), stop=(tt == ET - 1))
                for qc in range(2):
                    cs = slice(qc * 512, (qc + 1) * 512)
                    nc.vector.tensor_copy(out=den_h[:, cs], in_=po[qc][64:65, :])
                for qc in range(2):
                    cs = slice(qc * 512, (qc + 1) * 512)
                    nc.vector.tensor_copy(out=oTp[hsub, cs], in_=po[qc][0:64, :])
                # 1/den = exp(-ln(den)) on ACT (same table set as softmax exp)
                lnd = statp.tile([1, S], f32, tag=f"lnd{h}", name=f"lnd_{p}_{h}",
                                 bufs=2)
                nc.scalar.activation(out=lnd[:], in_=den_h[:], func=AF.Ln,
                                     scale=1.0)
                nc.scalar.activation(out=recp[h][:], in_=lnd[:], func=AF.Exp,
                                     scale=-1.0)

            ot_bf = otp.tile([128, S], bf16, tag=f"ob{p}", name=f"ob_{p}")
            for qc in range(2):
                cs = slice(qc * 512, (qc + 1) * 512)
                pr = pav.tile([128, 512], f32, tag="av", name=f"pr_{p}_{qc}")
                nc.tensor.matmul(pr[:], sel_h[0][:], recp[0][:, cs],
                                 start=True, stop=False)
                nc.tensor.matmul(pr[:], sel_h[1][:], recp[1][:, cs],
                                 start=False, stop=True)
                nc.vector.tensor_mul(ot_bf[:, cs], oTp[:, cs], pr[:])
            oT.append(ot_bf)

        # ---------- phase 3: partial out projection ----------
        for et in range(ET):
            for qc in range(2):
                cs = slice(qc * 512, (qc + 1) * 512)
                po = pmm.tile([128, 512], f32, tag="mm", name=f"po_{et}_{qc}")
                for ft in range(4):
                    nc.tensor.matmul(po[:], wo_t[ft][:, et * 128:(et + 1) * 128],
                                     oT[ft][:, cs],
                                     start=(ft == 0), stop=(ft == 3))
                ot = outp.tile([128, 512], bf16, tag="out", name=f"o_{et}_{qc}")
                if et % 2 == 0:
                    nc.vector.tensor_copy(out=ot[:], in_=po[:])
                else:
                    nc.scalar.activation(out=ot[:], in_=po[:], func=AF.Identity,
                                         scale=1.0)
                eng = nc.sync if (et + qc) % 2 == 0 else nc.gpsimd
                eng.dma_start(out=outT_d[et * 128:(et + 1) * 128, cs], in_=ot[:])

        lxp.release()
        outp.release()
        otp.release()
        attnp.release()
        qkvp.release()
        wsp.release()
        sqp.release()
        bcp.release()
        statp.release()
        consts.release()
        pav.release()
        psc.release()
        pmm.release()

    nc.compile()
    return nc


def _build_launch2():
    nc = bacc.Bacc("TRN2", target_bir_lowering=False, debug=False, num_devices=NCORES)

    toksT_d = nc.dram_tensor("toksT", [E, C], bf16, kind="ExternalInput").ap()
    w1_d = nc.dram_tensor("w1", [E, F], bf16, kind="ExternalInput").ap()
    w2_d = nc.dram_tensor("w2", [F, E], bf16, kind="ExternalInput").ap()
    b1_d = nc.dram_tensor("b1", [F, 1], f32, kind="ExternalInput").ap()
    b2_d = nc.dram_tensor("b2", [E, 1], f32, kind="ExternalInput").ap()
    outT_d = nc.dram_tensor("outT", [E, C], bf16, kind="ExternalOutput").ap()

    with tile.TileContext(nc) as tc:
        with (
            tc.tile_pool(name="consts", bufs=1) as consts,
            tc.tile_pool(name="tok", bufs=1) as tokp,
            tc.tile_pool(name="hp", bufs=1) as hp,
            tc.tile_pool(name="ws", bufs=6) as wsp,
            tc.tile_pool(name="outs", bufs=3) as outs,
            tc.tile_pool(name="pg1", bufs=4, space="PSUM") as pg1,
            tc.tile_pool(name="pg2", bufs=4, space="PSUM") as pg2,
        ):
            b1_sb = consts.tile([128, FT], f32, tag="b1")
            nc.sync.dma_start(out=b1_sb[:], in_=b1_d.rearrange("(a p) o -> p (a o)", p=128))
            b2_sb = consts.tile([128, ET], f32, tag="b2")
            nc.sync.dma_start(out=b2_sb[:], in_=b2_d.rearrange("(a p) o -> p (a o)", p=128))
            dust = consts.tile([1, 1], f32, tag="dust")
            nc.scalar.activation(out=dust[:], in_=b1_sb[0:1, 0:1], func=_GELU,
                                 scale=1.0)

            tokbig = tokp.tile([128, ET * C], bf16, tag="t", name="toks")
            tok_r = tokbig[:].rearrange("p (a c) -> p a c", a=ET)
            tokd_r = toksT_d.rearrange("(a p) c -> p a c", p=128)
            for ci, (off, w) in enumerate(CT):
                eng = (nc.sync, nc.scalar, nc.gpsimd)[ci % 3]
                eng.dma_start(out=tok_r[:, :, off:off + w],
                              in_=tokd_r[:, :, off:off + w])
            toks = [tokbig[:, i * C:(i + 1) * C] for i in range(ET)]

            hbf = []
            for ft in range(FT):
                hbf.append(hp.tile([128, C], bf16, tag=f"h{ft}", name=f"hbf{ft}"))

            # GEMM1: hT = gelu(w1.T @ toksT + b1)
            # weight DMAs pull [2 kt x 512 cols] per transfer (four ft tiles)
            w1_r = w1_d.rearrange("(a p) c -> p a c", p=128)
            for ftp in range(FT // 4):
                blks = []
                for kj in range(ET // 2):
                    wt = wsp.tile([128, 2 * 512], bf16, tag="w1",
                                  name=f"w1_{ftp}_{kj}", bufs=10)
                    eng = (nc.sync, nc.gpsimd, nc.scalar)[kj % 3]
                    eng.dma_start(
                        out=wt[:].rearrange("p (a c) -> p a c", a=2),
                        in_=w1_r[:, 2 * kj:2 * kj + 2,
                                 ftp * 512:(ftp + 1) * 512])
                    blks.append(wt)
                for sub in range(4):
                    ft = ftp * 4 + sub
                    ps = [pg1.tile([128, w], f32, tag="g1", name=f"pg1_{ft}_{ci}")
                          for ci, (off, w) in enumerate(CT)]
                    for kt in range(ET):
                        wv = blks[kt // 2][:, (kt % 2) * 512 + sub * 128:
                                           (kt % 2) * 512 + (sub + 1) * 128]
                        for ci, (off, w) in enumerate(CT):
                            nc.tensor.matmul(ps[ci][:], wv,
                                             toks[kt][:, off:off + w],
                                             start=(kt == 0), stop=(kt == ET - 1))
                    for ci, (off, w) in enumerate(CT):
                        nc.scalar.activation(out=hbf[ft][:, off:off + w], in_=ps[ci][:],
                                             func=_GELU, bias=b1_sb[:, ft:ft + 1],
                                             scale=1.0)

            # GEMM2: outT = w2.T @ hT + b2
            # weight blocks [128, 512] cover four et tiles, kept resident across
            # the et accumulations
            w2_r = w2_d.rearrange("(a p) c -> p a c", p=128)
            for etp in range(ET // 4):
                blks = []
                for fj in range(FT // 2):
                    wt = wsp.tile([128, 2 * 512], bf16, tag="w2",
                                  name=f"w2_{etp}_{fj}", bufs=20)
                    eng = (nc.sync, nc.gpsimd, nc.scalar)[fj % 3]
                    eng.dma_start(
                        out=wt[:].rearrange("p (a c) -> p a c", a=2),
                        in_=w2_r[:, 2 * fj:2 * fj + 2,
                                 etp * 512:(etp + 1) * 512])
                    blks.append(wt)
                for sub in range(4):
                    et = etp * 4 + sub
                    ps = [pg2.tile([128, w], f32, tag="g2", name=f"pg2_{et}_{ci}")
                          for ci, (off, w) in enumerate(CT)]
                    for ft in range(FT):
                        wv = blks[ft // 2][:, (ft % 2) * 512 + sub * 128:
                                           (ft % 2) * 512 + (sub + 1) * 128]
                        for ci, (off, w) in enumerate(CT):
                            nc.tensor.matmul(ps[ci][:], wv, hbf[ft][:, off:off + w],
                                             start=(ft == 0), stop=(ft == FT - 1))
                    for ci, (off, w) in enumerate(CT):
                        ot = outs.tile([128, 512], bf16, tag="ot", name=f"ot_{et}_{ci}")
                        nc.vector.tensor_scalar(out=ot[:, 0:w], in0=ps[ci][:],
                                                scalar1=b2_sb[:, et:et + 1],
                                                scalar2=None, op0=ALU.add)
                        eng = nc.sync if (et + ci) % 2 == 0 else nc.gpsimd
                        eng.dma_start(
                            out=outT_d[et * 128:(et + 1) * 128, off:off + w],
                            in_=ot[:, 0:w])

    nc.compile()
    return nc


def _get_programs():
    if "l1" not in _programs:
        _programs["l1"] = _build_launch1()
    if "l2" not in _programs:
        _programs["l2"] = _build_launch2()
    return _programs["l1"], _programs["l2"]


def _expert_ffn_host(toks, w1e, b1e, w2e, b2e):
    """Exact host fallback for capacity overflow (rare)."""
    from scipy.special import erf
    h = toks @ w1e + b1e
    h = 0.5 * h * (1.0 + erf(h / np.float32(np.sqrt(2.0))))
    return h.astype(np.float32) @ w2e + b2e


def kernel(**inputs):
    import ml_dtypes

    l1, l2 = _get_programs()

    x = np.ascontiguousarray(np.asarray(inputs["x"], dtype=np.float32))        # (S,B,E)
    in_w = np.asarray(inputs["in_proj_w"], dtype=np.float32)                   # (3E,E)
    in_b = np.asarray(inputs["in_proj_b"], dtype=np.float32)
    out_w = np.asarray(inputs["out_proj_w"], dtype=np.float32)                 # (E,E)
    out_b = np.asarray(inputs["out_proj_b"], dtype=np.float32)
    gate_w = np.asarray(inputs["gate_w"], dtype=np.float32)                    # (NE,E)
    w1 = np.asarray(inputs["w1"], dtype=np.float32)                            # (NE,E,F)
    b1 = np.asarray(inputs["b1"], dtype=np.float32)
    w2 = np.asarray(inputs["w2"], dtype=np.float32)                            # (NE,F,E)
    b2 = np.asarray(inputs["b2"], dtype=np.float32)
    ln1_g = np.asarray(inputs["ln1_g"], dtype=np.float32)
    ln1_b = np.asarray(inputs["ln1_b"], dtype=np.float32)
    ln2_g = np.asarray(inputs["ln2_g"], dtype=np.float32)
    ln2_b = np.asarray(inputs["ln2_b"], dtype=np.float32)

    bf = ml_dtypes.bfloat16
    col = lambda v: np.ascontiguousarray(v.reshape(-1, 1))

    # fold LN1 gain into the QKV weights; LN1 bias into the QKV bias
    in_w_f = in_w * ln1_g[None, :]                 # (3E, E)
    in_b_f = in_b + in_w @ ln1_b                   # (3E,)

    # per-core head-group weight packs: pair-major [q 128 | k 128 | v 128]
    # head-group g covers heads 8g..8g+7 -> feature rows 512g..512(g+1)
    wqkv_g, bqkv_g, woT_g = [], [], []
    for g in range(2):
        fs = slice(512 * g, 512 * (g + 1))
        wq = in_w_f[0 * E:1 * E][fs]               # (512, E)
        wk = in_w_f[1 * E:2 * E][fs]
        wv = in_w_f[2 * E:3 * E][fs]
        bq_ = in_b_f[0 * E:1 * E][fs]
        bk_ = in_b_f[1 * E:2 * E][fs]
        pack = np.empty((E, NP * 384), dtype=np.float32)
        bias = np.zeros((12 * 128, 1), dtype=np.float32)
        for p in range(NP):
            ps = slice(128 * p, 128 * (p + 1))
            pack[:, p * 384 + 0:p * 384 + 128] = wq[ps].T
            pack[:, p * 384 + 128:p * 384 + 256] = wk[ps].T
            pack[:, p * 384 + 256:p * 384 + 384] = wv[ps].T
            bias[p * 128:(p + 1) * 128, 0] = bq_[ps]
            bias[(4 + p) * 128:(5 + p) * 128, 0] = bk_[ps]
        wqkv_g.append(np.ascontiguousarray(pack.astype(bf)))
        bqkv_g.append(bias)
        woT_g.append(np.ascontiguousarray(out_w[:, fs].T.astype(bf)))  # (512, E)

    sel2 = np.zeros((2, 128), dtype=np.float32)
    sel2[0, 0:64] = 1.0
    sel2[1, 64:128] = 1.0
    ident = np.eye(128, dtype=np.float32).astype(bf)

    # v-bias folds into a constant added on the host:
    # (o/den + bv_g) @ Wo_g.T summed over g  ->  + bv @ Wo.T (+ out_b)
    bv = in_b_f[2 * E:3 * E]
    const_out = out_b + bv @ out_w.T               # (E,)

    # ---- launch 1 ----
    xT_b = [np.ascontiguousarray(x[:, b, :].T.astype(bf)) for b in range(B)]
    in_maps1 = []
    for c in range(NCORES):
        b, g = divmod(c, 2)
        in_maps1.append({
            "xT": xT_b[b],
            "wqkv": wqkv_g[g],
            "bqkv": bqkv_g[g],
            "woT": woT_g[g],
            "sel2": sel2,
            "ident": ident,
        })
    res1 = run_bass_kernel_spmd(l1, in_maps1, list(range(NCORES)))

    # combine partials + residual -> x2 [E, S, B]; then LN2 + gating on host
    x2_all = np.empty((E, S, B), dtype=np.float32)
    for b in range(B):
        x2_all[:, :, b] = (x[:, b, :].T
                           + res1.results[2 * b]["outT"].astype(np.float32)
                           + res1.results[2 * b + 1]["outT"].astype(np.float32)
                           + const_out[:, None])
    x2_flat = x2_all.reshape(E, N)                 # token n = s*B + b

    mu = x2_flat.mean(axis=0)
    var = x2_flat.var(axis=0)
    h2_flat = ((x2_flat - mu) / np.sqrt(var + 1e-5)) * ln2_g[:, None] + ln2_b[:, None]
    h2_flat = h2_flat.astype(np.float32)

    # ---- host gating: softmax over NE logits, top-2 renormalized ----
    logits = gate_w @ h2_flat                      # (NE, N)
    logits -= logits.max(axis=0, keepdims=True)
    p = np.exp(logits)
    p /= p.sum(axis=0, keepdims=True)
    ar = np.arange(N)
    i1 = np.argmax(p, axis=0)
    v1 = p[i1, ar]
    pm = p.copy()
    pm[i1, ar] = -1.0
    i2 = np.argmax(pm, axis=0)
    v2 = p[i2, ar]
    gsum = v1 + v2
    gate1 = v1 / gsum
    gate2 = v2 / gsum

    h2_bf = h2_flat.astype(bf)
    idx_list, gates_list, ov_list = [], [], []
    in_maps2 = []
    for e in range(NE):
        sel = np.where((i1 == e) | (i2 == e))[0]
        ge = np.where(i1[sel] == e, gate1[sel], gate2[sel]).astype(np.float32)
        ov = None
        if len(sel) > C:
            ov = (sel[C:], ge[C:])
            sel, ge = sel[:C], ge[:C]
        idx_list.append(sel)
        gates_list.append(ge)
        ov_list.append(ov)
        toksT = np.zeros((E, C), dtype=bf)
        toksT[:, :len(sel)] = h2_bf[:, sel]
        in_maps2.append({
            "toksT": toksT,
            "w1": w1[e].astype(bf),
            "w2": w2[e].astype(bf),
            "b1": col(b1[e]),
            "b2": col(b2[e]),
        })
    res2 = run_bass_kernel_spmd(l2, in_maps2, list(range(NCORES)))

    # ---- combine ----
    out_flat = x2_flat
    for e in range(NE):
        sel, ge = idx_list[e], gates_list[e]
        out_flat[:, sel] += (res2.results[e]["outT"][:, :len(sel)].astype(np.float32)
                             * ge[None, :])
        if ov_list[e] is not None:
            osel, oge = ov_list[e]
            oo = _expert_ffn_host(h2_flat[:, osel].T, w1[e], b1[e], w2[e], b2[e])
            out_flat[:, osel] += oo.T * oge[None, :]

    return np.ascontiguousarray(
        out_flat.reshape(E, S, B).transpose(1, 2, 0)).astype(np.float32)
